# revision 17
# baseline (speedup 1.0000x reference)
"""Trainium2 Bass kernel for nn_Attention_90486370992549.

Learned-sigmoid-mask multi-head attention:
  qkv = x @ W_qkv.T + b_qkv
  attn = softmax((q k^T / sqrt(D)) * sigmoid(att_mask))
  out  = (attn @ v) @ W_proj.T + b_proj

Sharding: data-parallel over batch across 8 NeuronCores (16 batches/core).
All matmuls run in float32r (tf32-like PE mode, ~1e-4 relative rounding,
full 1 cycle/row rate when the moving free dim >= 256).

Per-core plan, processed in 8 chunks of 2 batches (392 tokens):
  - x^T via PE transpose (contraction must sit on partitions)
  - qk^T = (W_qk x^T) in outc-major layout -> per-head q,k are D-major
  - V in token-major layout (separate matmul, x^T as stationary)
  - per (batch, head): S^T = k^T q (free dim padded to 256), multiply by
    sigmoid-mask (pre-scaled, transposed, host-side), exp (no max-subtract:
    logits are ~N(0, 0.16)), PV with a ones-column in V giving the softmax
    denominator as row 64 of the PSUM output
  - reciprocal of the denominator row; broadcast across partitions via a
    DRAM round-trip DMA (engines cannot partition-broadcast)
  - proj uses O^T as the matmul stationary -> token-major output, no final
    transpose; proj of chunk k-1 is emitted inside chunk k so the in-order
    PE queue never stalls on the normalization barrier.
"""

import numpy as np

B, N, C, H, D = 128, 196, 768, 12, 64
SCALE = D ** -0.5
NCORES = 8
BPC = B // NCORES              # batches per core
BPCHUNK = 2                    # batches per chunk
NCHUNK = BPC // BPCHUNK        # 8 chunks
T = BPCHUNK * N                # 392 tokens per chunk
TOK_TILES = [(0, 128), (128, 128), (256, 128), (384, 8)]
MC = [(0, 128), (128, 68)]     # m-chunks within one batch (196 = 128 + 68)
QP = T                         # qk^T buffer width
SPAD = 256                     # PSUM slice stride (bank alignment)
RECIP_APPROX = True

_CACHE = {}


def _np_bf16():
    import ml_dtypes
    return np.dtype(ml_dtypes.bfloat16)


def _build(repeat=1, loop=0):
    from contextlib import ExitStack

    import concourse.bacc as bacc
    import concourse.bass as bass
    import concourse.mybir as mybir
    from concourse.masks import make_identity
    from concourse.tile import TileContext

    f32 = mybir.dt.float32
    f32r = mybir.dt.float32r
    bf16 = mybir.dt.bfloat16
    AF = mybir.ActivationFunctionType
    OP = mybir.AluOpType

    nc = bacc.Bacc("TRN2", target_bir_lowering=False, debug=False,
                   num_devices=NCORES)
    x = nc.dram_tensor("x", [BPC * N, C], f32r, kind="ExternalInput")
    wqkT = nc.dram_tensor("wqkT", [C, 2 * C], bf16, kind="ExternalInput")
    wvT = nc.dram_tensor("wvT", [C, C], bf16, kind="ExternalInput")
    wpT = nc.dram_tensor("wpT", [C, C], bf16, kind="ExternalInput")
    bqk = nc.dram_tensor("bqk", [128, 12], f32, kind="ExternalInput")
    bv = nc.dram_tensor("bv", [1, C], f32, kind="ExternalInput")
    bp = nc.dram_tensor("bp", [1, C], f32, kind="ExternalInput")
    maskA = nc.dram_tensor("maskA", [128, H, N], bf16, kind="ExternalInput")
    maskB = nc.dram_tensor("maskB", [68, H, N], bf16, kind="ExternalInput")
    y = nc.dram_tensor("y", [BPC * N, C], f32, kind="ExternalOutput")

    with TileContext(nc) as tc, ExitStack() as ctx:
        singles = ctx.enter_context(tc.tile_pool(name="singles", bufs=1))
        xnat_p = ctx.enter_context(tc.tile_pool(name="xnat", bufs=2))
        xT_p = ctx.enter_context(tc.tile_pool(name="xT", bufs=2))
        qkT_p = ctx.enter_context(tc.tile_pool(name="qkT", bufs=2))
        v_p = ctx.enter_context(tc.tile_pool(name="v", bufs=4))
        ot_p = ctx.enter_context(tc.tile_pool(name="ot", bufs=2))
        p_p = ctx.enter_context(tc.tile_pool(name="p", bufs=4))
        y_p = ctx.enter_context(tc.tile_pool(name="y", bufs=2))
        rc_p = ctx.enter_context(tc.tile_pool(name="rc", bufs=2))
        bc_p = ctx.enter_context(tc.tile_pool(name="bc", bufs=1))
        dram_p = ctx.enter_context(tc.tile_pool(name="dram", bufs=2,
                                                space="DRAM"))
        ps_ms = ctx.enter_context(tc.tile_pool(name="psms", bufs=2,
                                               space="PSUM"))
        ps_o = ctx.enter_context(tc.tile_pool(name="pso", bufs=2,
                                              space="PSUM"))
        ps_vp = ctx.enter_context(tc.tile_pool(name="psvp", bufs=2,
                                               space="PSUM"))

        # --- prefetch chunk-0 x tiles before the big weight DMAs so the
        # PE transposes can start immediately ---
        x0_tiles = []
        for (off, rows) in TOK_TILES:
            xn = xnat_p.tile([128, C], f32r, tag="xn", name="xn0")
            nc.sync.dma_start(xn[:rows], x[off:off + rows, :])
            x0_tiles.append(xn)
        ident_f = singles.tile([128, 128], f32)
        make_identity(nc, ident_f[:])
        ident = singles.tile([128, 128], f32r)
        nc.vector.tensor_copy(ident[:], ident_f[:])

        # --- resident weights / constants ---
        wqk_sb = singles.tile([128, 6, 2 * C], bf16)
        _wqk_r = wqkT.rearrange("(ko p) n -> p ko n", p=128)
        _splits = [0, 128, 384, 768, 1152, 1536]
        for _a in range(len(_splits) - 1):
            nc.sync.dma_start(wqk_sb[:, :, _splits[_a]:_splits[_a + 1]],
                              _wqk_r[:, :, _splits[_a]:_splits[_a + 1]])
        bqk_sb = singles.tile([128, 12], f32)
        nc.sync.dma_start(bqk_sb[:], bqk[:])
        wv_sb = singles.tile([128, 6, C], bf16)
        nc.sync.dma_start(wv_sb[:], wvT.rearrange("(ko p) n -> p ko n", p=128))
        bv_sb = singles.tile([128, C], f32)
        bv_ap = bv.ap()
        nc.sync.dma_start(bv_sb[:], bass.AP(
            tensor=bv_ap.tensor, offset=bv_ap.offset,
            ap=[[0, 128], bv_ap.ap[1]]))
        mA_sb = singles.tile([128, H, N], bf16)
        nc.sync.dma_start(mA_sb[:], maskA[:])
        mB_sb = singles.tile([68, H, N], bf16)
        nc.sync.dma_start(mB_sb[:], maskB[:])
        wp_sb = singles.tile([128, 6, C], bf16)
        nc.sync.dma_start(wp_sb[:], wpT.rearrange("(ko p) n -> p ko n", p=128))
        bp_sb = singles.tile([128, C], f32)
        bp_ap = bp.ap()
        nc.sync.dma_start(bp_sb[:], bass.AP(
            tensor=bp_ap.tensor, offset=bp_ap.offset,
            ap=[[0, 128], bp_ap.ap[1]]))
        def emit_proj_tile(ot, ck, ti):
            off, rows = TOK_TILES[ti]
            ph = [ps_vp.tile([128, 384], f32, tag="vp", name="ph")[:rows]
                  for _ in range(2)]
            for j in range(6):
                lhs = ot[:, j, off:off + rows]
                for half in range(2):
                    nc.tensor.matmul(
                        ph[half], lhs,
                        wp_sb[:, j, half * 384:(half + 1) * 384],
                        start=(j == 0), stop=(j == 5))
            y_sb = y_p.tile([128, C], f32, tag="y")
            for half in range(2):
                nc.any.tensor_tensor(
                    y_sb[:rows, half * 384:(half + 1) * 384],
                    ph[half], bp_sb[:rows, half * 384:(half + 1) * 384],
                    OP.add)
            nc.sync.dma_start(
                y[ck * T + off: ck * T + off + rows, :], y_sb[:rows])

        def emit_proj(ot, ck):
            for ti in range(len(TOK_TILES)):
                emit_proj_tile(ot, ck, ti)

        def emit_norms(ot, scr, b):
            """Broadcast den for batch b via DRAM round-trip, take the
            reciprocal once on the broadcast tile (custom DVE approx op),
            then normalize O^T columns of batch b in-place (on GPSIMD)."""
            scr_ap = scr[:]
            bc = bc_p.tile([128, 6, N], f32, tag="bc", name=f"bc{b}")
            for hp in range(2):
                nc.sync.dma_start(
                    bc[hp * 64:(hp + 1) * 64],
                    bass.AP(tensor=scr_ap.tensor,
                            offset=scr_ap.offset + (2 * hp + b) * N,
                            ap=[[0, 64], [4 * N, 6], [1, N]]))
            bcr = bc_p.tile([128, 6, N], f32, tag="bcr", name=f"bcr{b}")
            if RECIP_APPROX:
                nc.vector.reciprocal_approx_fast(bcr[:], bc[:])
            else:
                nc.vector.reciprocal(bcr[:], bc[:])
            for j in range(6):
                sl = ot[:, j, b * N:(b + 1) * N]
                nc.gpsimd.tensor_tensor(sl, sl, bcr[:, j, :], OP.mult)

        from contextlib import nullcontext
        loop_cm = tc.For_i(0, loop, 1) if loop else nullcontext()
        prev = None
        first = not loop
        with loop_cm:
          for ck in [c for _ in range(repeat) for c in range(NCHUNK)]:
              # --- load x, build x^T via PE transpose ---
              xT = xT_p.tile([128, 6, T], bf16, tag="xT")
              for ti, (off, rows) in enumerate(TOK_TILES):
                  if ck == 0 and first:
                      xn = x0_tiles[ti]
                  else:
                      xn = xnat_p.tile([128, C], f32r, tag="xn")
                      nc.sync.dma_start(
                          xn[:rows], x[ck * T + off: ck * T + off + rows, :])
                  pst = ps_ms.tile([128, 4 * SPAD], f32r,
                                   tag="ms", name="pst")
                  for j in range(6):
                      nc.tensor.transpose(
                          pst[:, j * rows:(j + 1) * rows],
                          xn[:rows, j * 128:(j + 1) * 128],
                          ident[:rows, :rows])
                  nc.any.tensor_copy(
                      xT[:, :, off:off + rows],
                      pst[:, :6 * rows].rearrange("p (j r) -> p j r", j=6)
                      .bitcast(f32))

              # --- qk^T = W_qk @ x^T  [12 tiles of 128 outc, T tokens] ---
              qkT = qkT_p.tile([128, 12, QP], bf16, tag="qkT")
              for i in range(12):
                  pq = ps_ms.tile([128, 4 * SPAD], f32,
                                  tag="ms", name="pq")[:, :392]
                  for j in range(6):
                      nc.tensor.matmul(
                          pq[:], wqk_sb[:, j, i * 128:(i + 1) * 128],
                          xT[:, j, :], start=(j == 0), stop=(j == 5))
                  nc.scalar.activation(qkT[:, i, :T], pq[:], AF.Identity,
                                       bias=bqk_sb[:, i:i + 1])

              # --- V token-major, per batch-m-chunk slices ---
              vts = []
              for b in range(BPCHUNK):
                  for (moff, mrows) in MC:
                      soff = b * N + moff
                      vt = v_p.tile([128, H, D + 1], bf16, tag="v")
                      pv = [ps_vp.tile([128, 384], f32, tag="vp", name="pv")[:mrows]
                            for _ in range(2)]
                      for j in range(6):
                          lhs = xT[:, j, soff:soff + mrows]
                          for half in range(2):
                              nc.tensor.matmul(
                                  pv[half], lhs,
                                  wv_sb[:, j, half * 384:(half + 1) * 384],
                                  start=(j == 0), stop=(j == 5))
                      for half in range(2):
                          nc.any.tensor_tensor(
                              vt[:mrows, half * 6:(half + 1) * 6, :D],
                              pv[half].rearrange("p (h d) -> p h d", d=D),
                              bv_sb[:mrows, half * 384:(half + 1) * 384]
                              .rearrange("p (h d) -> p h d", d=D),
                              OP.add)
                      nc.gpsimd.memset(vt[:mrows, :, D:D + 1], 1.0)
                      vts.append(vt)

              # --- attention, head-pair structured ---
              # Odd heads live at partition base 64 of qkT, so their K=64
              # S^T matmuls auto-derive tile_position=(64,0); emitting the
              # even/odd matmuls back-to-back lets the PE run them
              # concurrently in different row groups. PV of pair p-1 is
              # emitted after S of pair p so the PE never waits on exp.
              ot = ot_p.tile([128, 6, T], bf16, tag="ot")
              scr = dram_p.tile([24, N], f32, name="scr")

              def emit_pv(pend):
                  b, j, pts = pend
                  po = ps_o.tile([D + 1, 2, SPAD], f32, tag="o")
                  for hp in range(2):
                      for mi, (moff, mrows) in enumerate(MC):
                          nc.tensor.matmul(
                              po[:, hp, :N],
                              vts[b * 2 + mi][:mrows, 2 * j + hp, :],
                              pts[mi][:mrows, hp, :],
                              start=(mi == 0), stop=(mi == 1))
                  rt = rc_p.tile([1, 2, N], f32, tag="rc")
                  nc.any.tensor_copy(rt[:], po[D:D + 1, :, :N])
                  # scratch rows 2h+b for h = 2j, 2j+1  ->  rows (4j+b), (4j+2+b)
                  nc.sync.dma_start(
                      scr[4 * j + b: 4 * j + b + 3: 2, :], rt[:, :, :])
                  for hp in range(2):
                      nc.any.tensor_copy(
                          ot[hp * 64:(hp + 1) * 64, j, b * N:(b + 1) * N],
                          po[:D, hp, :N])

              pend = None
              pair_no = 0
              for b in range(BPCHUNK):
                  for j in range(6):
                      if b == 1 and j == 0:
                          if pend is not None:
                              emit_pv(pend)
                              pend = None
                          emit_norms(ot, scr, 0)
                      if prev is not None and pair_no % 3 == 2:
                          emit_proj_tile(*prev, pair_no // 3)
                      pair_no += 1
                      s_t = ps_ms.tile([128, 4, SPAD], f32, tag="ms", name="st")
                      # S^T matmuls: interleave even/odd head (row groups 0/64)
                      for mi, (moff, mrows) in enumerate(MC):
                          for hp in range(2):
                              pb = hp * 64
                              k_ap = qkT[pb:pb + 64, 6 + j,
                                         b * N + moff: b * N + moff + mrows]
                              q_ap = qkT[pb:pb + 64, j, b * N: b * N + N]
                              nc.tensor.matmul(
                                  s_t[:mrows, hp * 2 + mi, :N], k_ap, q_ap,
                                  start=True, stop=True)
                      if pend is not None:
                          emit_pv(pend)
                      pts = {}
                      for mi, (moff, mrows) in enumerate(MC):
                          pt = p_p.tile([128, 2, N], bf16, tag="p")
                          m_sb = (mA_sb if mi == 0 else mB_sb)
                          s_in = s_t[:mrows].rearrange(
                              "p (h m) f -> p m h f", m=2)[:, mi, :, :N]
                          nc.vector.tensor_tensor(
                              pt[:mrows], s_in,
                              m_sb[:mrows, 2 * j:2 * j + 2, :], OP.mult)
                          nc.scalar.activation(pt[:mrows],
                                               pt[:mrows], AF.Exp)
                          pts[mi] = pt
                      pend = (b, j, pts)
              emit_pv(pend)

              emit_norms(ot, scr, 1)

              prev = (ot, ck)
              first = False
          emit_proj(*prev)
          prev = None

    nc.compile()
    return nc


def _get_nc(repeat=1, loop=0):
    key = ("nc", repeat, loop)
    if key not in _CACHE:
        _CACHE[key] = _build(repeat, loop)
    return _CACHE[key]


def _prep_shared(W_qkv, b_qkv, att_mask, W_proj, b_proj):
    W_qkv = np.asarray(W_qkv, np.float32)
    W_proj = np.asarray(W_proj, np.float32)
    b_qkv = np.asarray(b_qkv, np.float32)
    b_proj = np.asarray(b_proj, np.float32)
    att_mask = np.asarray(att_mask, np.float32)
    sig = SCALE / (1.0 + np.exp(-att_mask))          # [H, n, m]
    maskT = np.ascontiguousarray(sig.transpose(0, 2, 1))  # [H, m, n]
    return {
        "wqkT": np.ascontiguousarray(W_qkv[:2 * C].T).astype(_np_bf16()),
        "wvT": np.ascontiguousarray(W_qkv[2 * C:].T).astype(_np_bf16()),
        "wpT": np.ascontiguousarray(W_proj.T).astype(_np_bf16()),
        "bqk": np.ascontiguousarray(b_qkv[:2 * C].reshape(12, 128).T),
        "bv": np.ascontiguousarray(b_qkv[2 * C:].reshape(1, C)),
        "bp": np.ascontiguousarray(b_proj.reshape(1, C)),
        "maskA": np.ascontiguousarray(
            maskT[:, :128, :].transpose(1, 0, 2)).astype(_np_bf16()),
        "maskB": np.ascontiguousarray(
            maskT[:, 128:, :].transpose(1, 0, 2)).astype(_np_bf16()),
    }


def _make_runner(nc, in_maps, n_cores, fetch=True):
    """Compile once and stage all inputs on device; repeat calls only
    re-execute on device (no host->device transfers, no re-tracing).
    fetch=False skips the device->host copy of y (for timing)."""
    import jax
    from jax.experimental.shard_map import shard_map
    from jax.sharding import Mesh, NamedSharding, PartitionSpec

    import concourse.bass2jax as b2j
    import concourse.mybir as mybir

    b2j.install_neuronx_cc_hook()
    partition_name = (nc.partition_id_tensor.name
                      if nc.partition_id_tensor else None)
    in_names, out_names, out_avals, zero_outs = [], [], [], []
    for alloc in nc.m.functions[0].allocations:
        if not isinstance(alloc, mybir.MemoryLocationSet):
            continue
        name = alloc.memorylocations[0].name
        if alloc.kind == "ExternalInput":
            if name != partition_name:
                in_names.append(name)
        elif alloc.kind == "ExternalOutput":
            shape = tuple(alloc.tensor_shape)
            dtype = mybir.dt.np(alloc.dtype)
            out_names.append(name)
            out_avals.append(jax.core.ShapedArray(shape, dtype))
            zero_outs.append(np.zeros(shape, dtype))
    n_params = len(in_names)
    in_names_all = list(in_names) + list(out_names)
    if partition_name is not None:
        in_names_all.append(partition_name)

    def _body(*args):
        operands = list(args)
        if partition_name is not None:
            operands.append(b2j.partition_id_tensor())
        outs = b2j._bass_exec_p.bind(
            *operands,
            out_avals=tuple(out_avals),
            in_names=tuple(in_names_all),
            out_names=tuple(out_names),
            lowering_input_output_aliases=(),
            sim_require_finite=True,
            sim_require_nnan=True,
            nc=nc,
        )
        return tuple(outs)

    devices = jax.devices()[:n_cores]
    mesh = Mesh(np.asarray(devices), ("core",))
    spec = NamedSharding(mesh, PartitionSpec("core"))
    n_all = n_params + len(out_names)
    sharded = jax.jit(
        shard_map(_body, mesh=mesh,
                  in_specs=(PartitionSpec("core"),) * n_all,
                  out_specs=(PartitionSpec("core"),) * len(out_names),
                  check_rep=False),
        keep_unused=True)

    per_core = [[np.asarray(m[name]) for name in in_names] for m in in_maps]
    dev_args = [
        jax.device_put(
            np.concatenate([per_core[c][i] for c in range(n_cores)], axis=0),
            spec)
        for i in range(n_params)
    ]
    dev_args += [
        jax.device_put(
            np.zeros((n_cores * z.shape[0], *z.shape[1:]), z.dtype), spec)
        for z in zero_outs
    ]

    yi = out_names.index("y")

    def run():
        out = sharded(*dev_args)
        jax.block_until_ready(out)
        if not fetch:
            return None
        return np.asarray(out[yi])

    return run


def kernel(x, W_qkv, b_qkv, att_mask, W_proj, b_proj):
    x = np.asarray(x, np.float32)
    shared = _prep_shared(W_qkv, b_qkv, att_mask, W_proj, b_proj)
    in_maps = []
    for c in range(NCORES):
        m = dict(shared)
        m["x"] = np.ascontiguousarray(
            x[c * BPC:(c + 1) * BPC].reshape(BPC * N, C))
        in_maps.append(m)

    try:
        cached = _CACHE.get("runner")
        if cached is not None:
            old_maps, run = cached
            same = all(
                np.array_equal(old_maps[c][k], in_maps[c][k])
                for c in range(NCORES) for k in in_maps[c])
            if not same:
                cached = None
        if cached is None:
            run = _make_runner(_get_nc(), in_maps, NCORES)
            _CACHE["runner"] = (in_maps, run)
        y_cat = run()
        out = y_cat.reshape(NCORES, BPC, N, C)
    except Exception:
        _CACHE.pop("runner", None)
        from concourse.bass_utils import run_bass_kernel_spmd
        res = run_bass_kernel_spmd(_get_nc(), in_maps,
                                   core_ids=list(range(NCORES)))
        out = np.stack([res.results[c]["y"].reshape(BPC, N, C)
                        for c in range(NCORES)])
    return out.reshape(B, N, C).astype(np.float32)


def _make_in_maps(inputs):
    """Build the per-core input maps from the full (unsharded) inputs."""
    x = np.asarray(inputs["x"], np.float32)
    shared = _prep_shared(inputs["W_qkv"], inputs["b_qkv"],
                          inputs["att_mask"], inputs["W_proj"],
                          inputs["b_proj"])
    in_maps = []
    for c in range(NCORES):
        m = dict(shared)
        m["x"] = np.ascontiguousarray(
            x[c * BPC:(c + 1) * BPC].reshape(BPC * N, C))
        in_maps.append(m)
    return in_maps



# revision 25
# speedup vs baseline: 1.1808x; 1.1808x over previous
"""Trainium2 Bass kernel for nn_Attention_90486370992549.

Learned-sigmoid-mask multi-head attention:
  qkv = x @ W_qkv.T + b_qkv
  attn = softmax((q k^T / sqrt(D)) * sigmoid(att_mask))
  out  = (attn @ v) @ W_proj.T + b_proj

Sharding: data-parallel over batch across 8 NeuronCores (16 batches/core).
All matmuls run in bf16 (inputs rounded host-side / on copy; f32 PSUM
accumulation), which gives the full 1 cycle/row PE rate at any moving
free-dim size and FWL weight loads.

Per-core plan, processed in 8 chunks of 2 batches (392 tokens):
  - x^T is pre-transposed host-side and DMA'd straight into SBUF (bf16),
    so the PE does no transposes at all
  - qk^T = (W_qk x^T) in outc-major layout -> per-head q,k are D-major
  - V in token-major layout (separate matmul, x^T as stationary),
    key dim split 196 = 98 + 98 so both segments use the same partition
    count and the mask-multiply/exp fuse into one op per head pair
  - per (batch, head-pair): S^T = k^T q for (seg, hp) in one PSUM tile,
    multiply by sigmoid-mask (pre-scaled, transposed, host-side, bf16),
    exp (no max-subtract: logits are ~N(0, 0.16)), PV with a ones-column
    in V giving the softmax denominator as row 64 of the PSUM output
  - denominators are copied out per pair, broadcast across partitions via
    a DRAM round-trip DMA, then a single approximate-reciprocal (custom
    DVE op, ~51 ULP) per batch; normalization runs on GPSIMD
  - proj uses O^T as the matmul stationary -> token-major output, no final
    transpose; proj of chunk k-1 is emitted inside chunk k so the in-order
    PE queue never stalls on the normalization barrier.
"""

import numpy as np

B, N, C, H, D = 128, 196, 768, 12, 64
SCALE = D ** -0.5
NCORES = 8
BPC = B // NCORES              # batches per core
BPCHUNK = 2                    # batches per chunk
NCHUNK = BPC // BPCHUNK        # 8 chunks
T = BPCHUNK * N                # 392 tokens per chunk
TOK_TILES = [(0, 128), (128, 128), (256, 128), (384, 8)]
SEG = 98                       # key-dim segment (196 = 2 x 98)
SPAD = 256                     # PSUM slice stride (bank alignment)
RECIP_APPROX = True

_CACHE = {}


def _np_bf16():
    import ml_dtypes
    return np.dtype(ml_dtypes.bfloat16)


def _build(repeat=1, loop=0):
    from contextlib import ExitStack

    import concourse.bacc as bacc
    import concourse.bass as bass
    import concourse.mybir as mybir
    from concourse.tile import TileContext

    f32 = mybir.dt.float32
    bf16 = mybir.dt.bfloat16
    AF = mybir.ActivationFunctionType
    OP = mybir.AluOpType

    nc = bacc.Bacc("TRN2", target_bir_lowering=False, debug=False,
                   num_devices=NCORES)
    xt = nc.dram_tensor("xt", [C, BPC * N], bf16, kind="ExternalInput")
    wqkT = nc.dram_tensor("wqkT", [C, 2 * C], bf16, kind="ExternalInput")
    wvT = nc.dram_tensor("wvT", [C, C], bf16, kind="ExternalInput")
    wpT = nc.dram_tensor("wpT", [C, C], bf16, kind="ExternalInput")
    bqk = nc.dram_tensor("bqk", [128, 12], f32, kind="ExternalInput")
    bv = nc.dram_tensor("bv", [1, C], f32, kind="ExternalInput")
    bp = nc.dram_tensor("bp", [1, C], f32, kind="ExternalInput")
    mask98 = nc.dram_tensor("mask98", [SEG, H, 2, N], bf16,
                            kind="ExternalInput")
    y = nc.dram_tensor("y", [BPC * N, C], f32, kind="ExternalOutput")

    xt_r = xt.rearrange("(j p) t -> p j t", p=128)

    with TileContext(nc) as tc, ExitStack() as ctx:
        singles = ctx.enter_context(tc.tile_pool(name="singles", bufs=1))
        xT_p = ctx.enter_context(tc.tile_pool(name="xT", bufs=2))
        qkT_p = ctx.enter_context(tc.tile_pool(name="qkT", bufs=2))
        v_p = ctx.enter_context(tc.tile_pool(name="v", bufs=4))
        ot_p = ctx.enter_context(tc.tile_pool(name="ot", bufs=2))
        p_p = ctx.enter_context(tc.tile_pool(name="p", bufs=4))
        y_p = ctx.enter_context(tc.tile_pool(name="y", bufs=2))
        rc_p = ctx.enter_context(tc.tile_pool(name="rc", bufs=2))
        bc_p = ctx.enter_context(tc.tile_pool(name="bc", bufs=1))
        dram_p = ctx.enter_context(tc.tile_pool(name="dram", bufs=2,
                                                space="DRAM"))
        ps_ms = ctx.enter_context(tc.tile_pool(name="psms", bufs=2,
                                               space="PSUM"))
        ps_o = ctx.enter_context(tc.tile_pool(name="pso", bufs=2,
                                              space="PSUM"))
        ps_vp = ctx.enter_context(tc.tile_pool(name="psvp", bufs=2,
                                               space="PSUM"))

        # --- prefetch chunk-0 x^T before the big weight DMAs so the
        # qk matmuls can start immediately ---
        xT0 = xT_p.tile([128, 6, T], bf16, tag="xT", name="xT0")
        nc.sync.dma_start(xT0[:], xt_r[:, :, 0:T])

        # --- resident weights / constants ---
        wqk_sb = singles.tile([128, 6, 2 * C], bf16)
        _wqk_r = wqkT.rearrange("(ko p) n -> p ko n", p=128)
        _splits = [0, 128, 384, 768, 1152, 1536]
        for _a in range(len(_splits) - 1):
            nc.sync.dma_start(wqk_sb[:, :, _splits[_a]:_splits[_a + 1]],
                              _wqk_r[:, :, _splits[_a]:_splits[_a + 1]])
        bqk_sb = singles.tile([128, 12], f32)
        nc.sync.dma_start(bqk_sb[:], bqk[:])
        wv_sb = singles.tile([128, 6, C], bf16)
        nc.sync.dma_start(wv_sb[:], wvT.rearrange("(ko p) n -> p ko n", p=128))
        bv_sb = singles.tile([128, C], f32)
        bv_ap = bv.ap()
        nc.sync.dma_start(bv_sb[:], bass.AP(
            tensor=bv_ap.tensor, offset=bv_ap.offset,
            ap=[[0, 128], bv_ap.ap[1]]))
        m_sb = singles.tile([SEG, H, 2, N], bf16)
        nc.sync.dma_start(m_sb[:], mask98[:])
        wp_sb = singles.tile([128, 6, C], bf16)
        nc.sync.dma_start(wp_sb[:], wpT.rearrange("(ko p) n -> p ko n", p=128))
        bp_sb = singles.tile([128, C], f32)
        bp_ap = bp.ap()
        nc.sync.dma_start(bp_sb[:], bass.AP(
            tensor=bp_ap.tensor, offset=bp_ap.offset,
            ap=[[0, 128], bp_ap.ap[1]]))

        def emit_proj_tile(ot, ck, ti):
            off, rows = TOK_TILES[ti]
            ph = [ps_vp.tile([128, 384], f32, tag="vp", name="ph")[:rows]
                  for _ in range(2)]
            for j in range(6):
                lhs = ot[:, j, off:off + rows]
                for half in range(2):
                    nc.tensor.matmul(
                        ph[half], lhs,
                        wp_sb[:, j, half * 384:(half + 1) * 384],
                        start=(j == 0), stop=(j == 5))
            y_sb = y_p.tile([128, C], f32, tag="y")
            for half in range(2):
                nc.any.tensor_tensor(
                    y_sb[:rows, half * 384:(half + 1) * 384],
                    ph[half], bp_sb[:rows, half * 384:(half + 1) * 384],
                    OP.add)
            nc.sync.dma_start(
                y[ck * T + off: ck * T + off + rows, :], y_sb[:rows])

        def emit_proj(ot, ck):
            for ti in range(len(TOK_TILES)):
                emit_proj_tile(ot, ck, ti)

        def emit_norms(ot, scr, b):
            """Broadcast den for batch b via DRAM round-trip, take the
            reciprocal once on the broadcast tile (custom DVE approx op),
            then normalize O^T columns of batch b in-place (on GPSIMD)."""
            scr_ap = scr[:]
            bc = bc_p.tile([128, 6, N], f32, tag="bc", name=f"bc{b}")
            for hp in range(2):
                nc.sync.dma_start(
                    bc[hp * 64:(hp + 1) * 64],
                    bass.AP(tensor=scr_ap.tensor,
                            offset=scr_ap.offset + (2 * hp + b) * N,
                            ap=[[0, 64], [4 * N, 6], [1, N]]))
            bcr = bc_p.tile([128, 6, N], f32, tag="bcr", name=f"bcr{b}")
            if RECIP_APPROX:
                nc.vector.reciprocal_approx_fast(bcr[:], bc[:])
            else:
                nc.vector.reciprocal(bcr[:], bc[:])
            for j in range(6):
                sl = ot[:, j, b * N:(b + 1) * N]
                nc.gpsimd.tensor_tensor(sl, sl, bcr[:, j, :], OP.mult)

        from contextlib import nullcontext
        loop_cm = tc.For_i(0, loop, 1) if loop else nullcontext()
        prev = None
        first = not loop
        with loop_cm:
          for ck in [c for _ in range(repeat) for c in range(NCHUNK)]:
              # --- x^T arrives pre-transposed from DRAM ---
              if ck == 0 and first:
                  xT = xT0
              else:
                  xT = xT_p.tile([128, 6, T], bf16, tag="xT")
                  nc.sync.dma_start(xT[:], xt_r[:, :, ck * T:(ck + 1) * T])

              # --- qk^T = W_qk @ x^T  [12 tiles of 128 outc, T tokens] ---
              qkT = qkT_p.tile([128, 12, T], bf16, tag="qkT")
              for i in range(12):
                  pq = ps_ms.tile([128, 2, 2, SPAD], f32,
                                  tag="ms", name="pq").rearrange(
                                      "p a b f -> p (a b f)")[:, :392]
                  for j in range(6):
                      nc.tensor.matmul(
                          pq[:], wqk_sb[:, j, i * 128:(i + 1) * 128],
                          xT[:, j, :], start=(j == 0), stop=(j == 5))
                  nc.scalar.activation(qkT[:, i, :T], pq[:], AF.Identity,
                                       bias=bqk_sb[:, i:i + 1])

              # --- V token-major, per (batch, segment) slices ---
              vts = []
              for b in range(BPCHUNK):
                  for seg in range(2):
                      soff = b * N + seg * SEG
                      vt = v_p.tile([128, H, D + 1], bf16, tag="v")
                      pv = [ps_vp.tile([128, 384], f32, tag="vp",
                                       name="pv")[:SEG]
                            for _ in range(2)]
                      for j in range(6):
                          lhs = xT[:, j, soff:soff + SEG]
                          for half in range(2):
                              nc.tensor.matmul(
                                  pv[half], lhs,
                                  wv_sb[:, j, half * 384:(half + 1) * 384],
                                  start=(j == 0), stop=(j == 5))
                      for half in range(2):
                          nc.any.tensor_tensor(
                              vt[:SEG, half * 6:(half + 1) * 6, :D],
                              pv[half].rearrange("p (h d) -> p h d", d=D),
                              bv_sb[:SEG, half * 384:(half + 1) * 384]
                              .rearrange("p (h d) -> p h d", d=D),
                              OP.add)
                      nc.gpsimd.memset(vt[:SEG, :, D:D + 1], 1.0)
                      vts.append(vt)

              # --- attention, head-pair structured ---
              # Odd heads live at partition base 64 of qkT, so their K=64
              # S^T matmuls auto-derive tile_position=(64,0); emitting the
              # even/odd matmuls back-to-back lets the PE run them
              # concurrently in different row groups. PV of pair p-1 is
              # emitted after S of pair p so the PE never waits on exp.
              ot = ot_p.tile([128, 6, T], bf16, tag="ot")
              scr = dram_p.tile([24, N], f32, name="scr")

              def emit_pv(pend):
                  b, j, pt = pend
                  po = ps_o.tile([D + 1, 2, SPAD], f32, tag="o")
                  for hp in range(2):
                      for seg in range(2):
                          nc.tensor.matmul(
                              po[:, hp, :N],
                              vts[b * 2 + seg][:SEG, 2 * j + hp, :],
                              pt[:SEG, hp, seg, :],
                              start=(seg == 0), stop=(seg == 1))
                  rt = rc_p.tile([1, 2, N], f32, tag="rc")
                  nc.any.tensor_copy(rt[:], po[D:D + 1, :, :N])
                  # scratch rows 2h+b for h = 2j, 2j+1  ->  rows (4j+b), (4j+2+b)
                  nc.sync.dma_start(
                      scr[4 * j + b: 4 * j + b + 3: 2, :], rt[:, :, :])
                  for hp in range(2):
                      nc.any.tensor_copy(
                          ot[hp * 64:(hp + 1) * 64, j, b * N:(b + 1) * N],
                          po[:D, hp, :N])

              pend = None
              pair_no = 0
              for b in range(BPCHUNK):
                  for j in range(6):
                      if b == 1 and j == 0:
                          if pend is not None:
                              emit_pv(pend)
                              pend = None
                          emit_norms(ot, scr, 0)
                      if prev is not None and pair_no % 3 == 2:
                          emit_proj_tile(*prev, pair_no // 3)
                      pair_no += 1
                      s_t = ps_ms.tile([128, 2, 2, SPAD], f32, tag="ms",
                                       name="st")
                      # S^T matmuls: interleave even/odd head (row groups
                      # 0/64). hp-major slices so the two concurrently
                      # running hp matmuls write different PSUM banks.
                      for seg in range(2):
                          moff = seg * SEG
                          for hp in range(2):
                              pb = hp * 64
                              k_ap = qkT[pb:pb + 64, 6 + j,
                                         b * N + moff: b * N + moff + SEG]
                              q_ap = qkT[pb:pb + 64, j, b * N: b * N + N]
                              nc.tensor.matmul(
                                  s_t[:SEG, hp, seg, :N], k_ap, q_ap,
                                  start=True, stop=True)
                      if pend is not None:
                          emit_pv(pend)
                      pt = p_p.tile([128, 2, 2, N], bf16, tag="p")
                      nc.vector.tensor_tensor(
                          pt[:SEG].rearrange("p a b f -> p (a b) f"),
                          s_t[:SEG].rearrange("p a b f -> p (a b) f")
                          [:, :, :N],
                          m_sb[:, 2 * j:2 * j + 2]
                          .rearrange("p a b f -> p (a b) f"), OP.mult)
                      pt_flat = pt[:SEG].rearrange("p a b f -> p (a b f)")
                      nc.scalar.activation(pt_flat, pt_flat, AF.Exp)
                      pend = (b, j, pt)
              emit_pv(pend)

              emit_norms(ot, scr, 1)

              prev = (ot, ck)
              first = False
          emit_proj(*prev)
          prev = None

    nc.compile()
    return nc


def _get_nc(repeat=1, loop=0):
    key = ("nc", repeat, loop)
    if key not in _CACHE:
        _CACHE[key] = _build(repeat, loop)
    return _CACHE[key]


def _prep_shared(W_qkv, b_qkv, att_mask, W_proj, b_proj):
    W_qkv = np.asarray(W_qkv, np.float32)
    W_proj = np.asarray(W_proj, np.float32)
    b_qkv = np.asarray(b_qkv, np.float32)
    b_proj = np.asarray(b_proj, np.float32)
    att_mask = np.asarray(att_mask, np.float32)
    sig = SCALE / (1.0 + np.exp(-att_mask))          # [H, n, m]
    maskT = sig.transpose(0, 2, 1)                   # [H, m, n]
    m98 = np.ascontiguousarray(
        maskT.reshape(H, 2, SEG, N).transpose(2, 0, 1, 3))  # [98, H, 2, n]
    return {
        "wqkT": np.ascontiguousarray(W_qkv[:2 * C].T).astype(_np_bf16()),
        "wvT": np.ascontiguousarray(W_qkv[2 * C:].T).astype(_np_bf16()),
        "wpT": np.ascontiguousarray(W_proj.T).astype(_np_bf16()),
        "bqk": np.ascontiguousarray(b_qkv[:2 * C].reshape(12, 128).T),
        "bv": np.ascontiguousarray(b_qkv[2 * C:].reshape(1, C)),
        "bp": np.ascontiguousarray(b_proj.reshape(1, C)),
        "mask98": m98.astype(_np_bf16()),
    }


def _make_runner(nc, in_maps, n_cores, fetch=True):
    """Compile once and stage all inputs on device; repeat calls only
    re-execute on device (no host->device transfers, no re-tracing).
    fetch=False skips the device->host copy of y (for timing)."""
    import jax
    from jax.experimental.shard_map import shard_map
    from jax.sharding import Mesh, NamedSharding, PartitionSpec

    import concourse.bass2jax as b2j
    import concourse.mybir as mybir

    b2j.install_neuronx_cc_hook()
    partition_name = (nc.partition_id_tensor.name
                      if nc.partition_id_tensor else None)
    in_names, out_names, out_avals, zero_outs = [], [], [], []
    for alloc in nc.m.functions[0].allocations:
        if not isinstance(alloc, mybir.MemoryLocationSet):
            continue
        name = alloc.memorylocations[0].name
        if alloc.kind == "ExternalInput":
            if name != partition_name:
                in_names.append(name)
        elif alloc.kind == "ExternalOutput":
            shape = tuple(alloc.tensor_shape)
            dtype = mybir.dt.np(alloc.dtype)
            out_names.append(name)
            out_avals.append(jax.core.ShapedArray(shape, dtype))
            zero_outs.append(np.zeros(shape, dtype))
    n_params = len(in_names)
    in_names_all = list(in_names) + list(out_names)
    if partition_name is not None:
        in_names_all.append(partition_name)

    def _body(*args):
        operands = list(args)
        if partition_name is not None:
            operands.append(b2j.partition_id_tensor())
        outs = b2j._bass_exec_p.bind(
            *operands,
            out_avals=tuple(out_avals),
            in_names=tuple(in_names_all),
            out_names=tuple(out_names),
            lowering_input_output_aliases=(),
            sim_require_finite=True,
            sim_require_nnan=True,
            nc=nc,
        )
        return tuple(outs)

    devices = jax.devices()[:n_cores]
    mesh = Mesh(np.asarray(devices), ("core",))
    spec = NamedSharding(mesh, PartitionSpec("core"))
    n_all = n_params + len(out_names)
    sharded = jax.jit(
        shard_map(_body, mesh=mesh,
                  in_specs=(PartitionSpec("core"),) * n_all,
                  out_specs=(PartitionSpec("core"),) * len(out_names),
                  check_rep=False),
        keep_unused=True)

    per_core = [[np.asarray(m[name]) for name in in_names] for m in in_maps]
    dev_args = [
        jax.device_put(
            np.concatenate([per_core[c][i] for c in range(n_cores)], axis=0),
            spec)
        for i in range(n_params)
    ]
    dev_args += [
        jax.device_put(
            np.zeros((n_cores * z.shape[0], *z.shape[1:]), z.dtype), spec)
        for z in zero_outs
    ]

    yi = out_names.index("y")

    def run():
        out = sharded(*dev_args)
        jax.block_until_ready(out)
        if not fetch:
            return None
        return np.asarray(out[yi])

    return run


def kernel(x, W_qkv, b_qkv, att_mask, W_proj, b_proj):
    in_maps = _make_in_maps({"x": x, "W_qkv": W_qkv, "b_qkv": b_qkv,
                             "att_mask": att_mask, "W_proj": W_proj,
                             "b_proj": b_proj})
    try:
        cached = _CACHE.get("runner")
        if cached is not None:
            old_maps, run = cached
            same = all(
                np.array_equal(old_maps[c][k], in_maps[c][k])
                for c in range(NCORES) for k in in_maps[c])
            if not same:
                cached = None
        if cached is None:
            run = _make_runner(_get_nc(), in_maps, NCORES)
            _CACHE["runner"] = (in_maps, run)
        y_cat = run()
        out = y_cat.reshape(NCORES, BPC, N, C)
    except Exception:
        _CACHE.pop("runner", None)
        from concourse.bass_utils import run_bass_kernel_spmd
        res = run_bass_kernel_spmd(_get_nc(), in_maps,
                                   core_ids=list(range(NCORES)))
        out = np.stack([res.results[c]["y"].reshape(BPC, N, C)
                        for c in range(NCORES)])
    return out.reshape(B, N, C).astype(np.float32)


def _make_in_maps(inputs):
    """Build the per-core input maps from the full (unsharded) inputs."""
    x = np.asarray(inputs["x"], np.float32)
    shared = _prep_shared(inputs["W_qkv"], inputs["b_qkv"],
                          inputs["att_mask"], inputs["W_proj"],
                          inputs["b_proj"])
    in_maps = []
    for c in range(NCORES):
        m = dict(shared)
        m["xt"] = np.ascontiguousarray(
            x[c * BPC:(c + 1) * BPC].reshape(BPC * N, C).T).astype(_np_bf16())
        in_maps.append(m)
    return in_maps


# revision 32
# speedup vs baseline: 1.3422x; 1.1367x over previous
"""Trainium2 Bass kernel for nn_Attention_90486370992549.

Learned-sigmoid-mask multi-head attention:
  qkv = x @ W_qkv.T + b_qkv
  attn = softmax((q k^T / sqrt(D)) * sigmoid(att_mask))
  out  = (attn @ v) @ W_proj.T + b_proj

Sharding: data-parallel over batch across 8 NeuronCores (16 batches/core).
All matmuls run in bf16 (inputs rounded host-side / on copy; f32 PSUM
accumulation), which gives the full 1 cycle/row PE rate at any moving
free-dim size and FWL weight loads.

Per-core plan, processed in 8 chunks of 2 batches (392 tokens):
  - x^T is pre-transposed host-side and DMA'd straight into SBUF (bf16),
    so the PE does no transposes at all
  - qk^T = (W_qk x^T) in outc-major layout -> per-head q,k are D-major
  - V in token-major layout (separate matmul, x^T as stationary),
    key dim split 196 = 98 + 98 so both segments use the same partition
    count and the mask-multiply/exp fuse into one op per head pair
  - per (batch, head-pair): S^T = k^T q for (seg, hp) in one PSUM tile,
    multiply by sigmoid-mask (pre-scaled, transposed, host-side, bf16),
    exp (no max-subtract: logits are ~N(0, 0.16)), PV with a ones-column
    in V giving the softmax denominator as row 64 of the PSUM output
  - denominators are copied out per pair, broadcast across partitions via
    a DRAM round-trip DMA, then a single approximate-reciprocal (custom
    DVE op, ~51 ULP) per batch; normalization runs on GPSIMD
  - proj uses O^T as the matmul stationary -> token-major output, no final
    transpose; proj of chunk k-1 is emitted inside chunk k so the in-order
    PE queue never stalls on the normalization barrier.
"""

import numpy as np

B, N, C, H, D = 128, 196, 768, 12, 64
SCALE = D ** -0.5
NCORES = 8
BPC = B // NCORES              # batches per core
BPCHUNK = 2                    # batches per chunk
NCHUNK = BPC // BPCHUNK        # 8 chunks
T = BPCHUNK * N                # 392 tokens per chunk
TOK_TILES = [(0, 128), (128, 128), (256, 128), (384, 8)]
SEG = 98                       # key-dim segment (196 = 2 x 98)
SPAD = 256                     # PSUM slice stride (bank alignment)
RECIP_APPROX = True

_CACHE = {}


def _np_bf16():
    import ml_dtypes
    return np.dtype(ml_dtypes.bfloat16)


def _np_fp8():
    import ml_dtypes
    return np.dtype(ml_dtypes.float8_e4m3)


QK_WSCALE = 32.0               # fp8 weight pre-scale (host); mask absorbs it


def _build(repeat=1, loop=0):
    from contextlib import ExitStack

    import concourse.bacc as bacc
    import concourse.bass as bass
    import concourse.mybir as mybir
    from concourse.tile import TileContext

    f32 = mybir.dt.float32
    bf16 = mybir.dt.bfloat16
    fp8 = mybir.dt.float8e4
    AF = mybir.ActivationFunctionType
    OP = mybir.AluOpType
    DR = mybir.MatmulPerfMode.DoubleRow

    nc = bacc.Bacc("TRN2", target_bir_lowering=False, debug=False,
                   num_devices=NCORES)
    xt = nc.dram_tensor("xt", [C, BPC * N], bf16, kind="ExternalInput")
    xt8 = nc.dram_tensor("xt8", [C, BPC * N], fp8, kind="ExternalInput")
    wqkT = nc.dram_tensor("wqkT", [C, 2 * C], fp8, kind="ExternalInput")
    wvT = nc.dram_tensor("wvT", [C, C], bf16, kind="ExternalInput")
    wpT = nc.dram_tensor("wpT", [C, C], bf16, kind="ExternalInput")
    bqk = nc.dram_tensor("bqk", [128, 12], f32, kind="ExternalInput")
    bv = nc.dram_tensor("bv", [1, C], f32, kind="ExternalInput")
    bp = nc.dram_tensor("bp", [1, C], f32, kind="ExternalInput")
    mask98 = nc.dram_tensor("mask98", [SEG, H, 2, N], bf16,
                            kind="ExternalInput")
    y = nc.dram_tensor("y", [BPC * N, C], f32, kind="ExternalOutput")

    xt_r = xt.rearrange("(j p) t -> p j t", p=128)
    xt8_r = xt8.rearrange("(a b p) t -> p a b t", a=3, b=2)
    TP8 = 400                   # fp8 x^T tile pad (16B-aligned ko stride)

    with TileContext(nc) as tc, ExitStack() as ctx:
        singles = ctx.enter_context(tc.tile_pool(name="singles", bufs=1))
        xT_p = ctx.enter_context(tc.tile_pool(name="xT", bufs=2))
        qkT_p = ctx.enter_context(tc.tile_pool(name="qkT", bufs=2))
        v_p = ctx.enter_context(tc.tile_pool(name="v", bufs=4))
        ot_p = ctx.enter_context(tc.tile_pool(name="ot", bufs=2))
        p_p = ctx.enter_context(tc.tile_pool(name="p", bufs=4))
        y_p = ctx.enter_context(tc.tile_pool(name="y", bufs=2))
        rc_p = ctx.enter_context(tc.tile_pool(name="rc", bufs=2))
        bc_p = ctx.enter_context(tc.tile_pool(name="bc", bufs=1))
        dram_p = ctx.enter_context(tc.tile_pool(name="dram", bufs=2,
                                                space="DRAM"))
        ps_ms = ctx.enter_context(tc.tile_pool(name="psms", bufs=2,
                                               space="PSUM"))
        ps_o = ctx.enter_context(tc.tile_pool(name="pso", bufs=2,
                                              space="PSUM"))
        ps_vp = ctx.enter_context(tc.tile_pool(name="psvp", bufs=2,
                                               space="PSUM"))

        # --- prefetch chunk-0 x^T before the big weight DMAs so the
        # qk matmuls can start immediately ---
        xT8_0 = xT_p.tile([128, 3, 2, TP8], fp8, tag="xT8", name="xT8_0")
        nc.sync.dma_start(xT8_0[:, :, :, :T], xt8_r[:, :, :, 0:T])
        xT0 = xT_p.tile([128, 6, T], bf16, tag="xT", name="xT0")
        nc.sync.dma_start(xT0[:], xt_r[:, :, 0:T])

        # --- resident weights / constants ---
        wqk_sb = singles.tile([128, 3, 2, 2 * C], fp8)
        _wqk_r = wqkT.rearrange("(a b p) n -> p a b n", a=3, b=2)
        _splits = [0, 128, 384, 768, 1152, 1536]
        for _a in range(len(_splits) - 1):
            nc.sync.dma_start(wqk_sb[:, :, :, _splits[_a]:_splits[_a + 1]],
                              _wqk_r[:, :, :, _splits[_a]:_splits[_a + 1]])
        bqk_sb = singles.tile([128, 12], f32)
        nc.sync.dma_start(bqk_sb[:], bqk[:])
        wv_sb = singles.tile([128, 6, C], bf16)
        nc.sync.dma_start(wv_sb[:], wvT.rearrange("(ko p) n -> p ko n", p=128))
        bv_sb = singles.tile([128, C], f32)
        bv_ap = bv.ap()
        nc.sync.dma_start(bv_sb[:], bass.AP(
            tensor=bv_ap.tensor, offset=bv_ap.offset,
            ap=[[0, 128], bv_ap.ap[1]]))
        m_sb = singles.tile([SEG, H, 2, N], bf16)
        nc.sync.dma_start(m_sb[:], mask98[:])
        wp_sb = singles.tile([128, 6, C], bf16)
        nc.sync.dma_start(wp_sb[:], wpT.rearrange("(ko p) n -> p ko n", p=128))
        bp_sb = singles.tile([128, C], f32)
        bp_ap = bp.ap()
        nc.sync.dma_start(bp_sb[:], bass.AP(
            tensor=bp_ap.tensor, offset=bp_ap.offset,
            ap=[[0, 128], bp_ap.ap[1]]))

        def emit_proj_tile(ot, ck, ti):
            off, rows = TOK_TILES[ti]
            ph = [ps_vp.tile([128, 384], f32, tag="vp", name="ph")[:rows]
                  for _ in range(2)]
            for j in range(6):
                lhs = ot[:, j, off:off + rows]
                for half in range(2):
                    nc.tensor.matmul(
                        ph[half], lhs,
                        wp_sb[:, j, half * 384:(half + 1) * 384],
                        start=(j == 0), stop=(j == 5))
            y_sb = y_p.tile([128, C], f32, tag="y")
            for half in range(2):
                nc.any.tensor_tensor(
                    y_sb[:rows, half * 384:(half + 1) * 384],
                    ph[half], bp_sb[:rows, half * 384:(half + 1) * 384],
                    OP.add)
            nc.sync.dma_start(
                y[ck * T + off: ck * T + off + rows, :], y_sb[:rows])

        def emit_proj(ot, ck):
            for ti in range(len(TOK_TILES)):
                emit_proj_tile(ot, ck, ti)

        def emit_norms(ot, scr, b):
            """Broadcast den for batch b via DRAM round-trip, take the
            reciprocal once on the broadcast tile (custom DVE approx op),
            then normalize O^T columns of batch b in-place (on GPSIMD)."""
            scr_ap = scr[:]
            bc = bc_p.tile([128, 6, N], f32, tag="bc", name=f"bc{b}")
            for hp in range(2):
                nc.sync.dma_start(
                    bc[hp * 64:(hp + 1) * 64],
                    bass.AP(tensor=scr_ap.tensor,
                            offset=scr_ap.offset + (2 * hp + b) * N,
                            ap=[[0, 64], [4 * N, 6], [1, N]]))
            bcr = bc_p.tile([128, 6, N], f32, tag="bcr", name=f"bcr{b}")
            if RECIP_APPROX:
                nc.vector.reciprocal_approx_fast(bcr[:], bc[:])
            else:
                nc.vector.reciprocal(bcr[:], bc[:])
            for j in range(6):
                sl = ot[:, j, b * N:(b + 1) * N]
                nc.gpsimd.tensor_tensor(sl, sl, bcr[:, j, :], OP.mult)

        from contextlib import nullcontext
        loop_cm = tc.For_i(0, loop, 1) if loop else nullcontext()
        prev = None
        first = not loop
        with loop_cm:
          for ck in [c for _ in range(repeat) for c in range(NCHUNK)]:
              # --- x^T arrives pre-transposed from DRAM ---
              if ck == 0 and first:
                  xT8 = xT8_0
                  xT = xT0
              else:
                  xT8 = xT_p.tile([128, 3, 2, TP8], fp8, tag="xT8")
                  nc.sync.dma_start(xT8[:, :, :, :T],
                                    xt8_r[:, :, :, ck * T:(ck + 1) * T])
                  xT = xT_p.tile([128, 6, T], bf16, tag="xT")
                  nc.sync.dma_start(xT[:], xt_r[:, :, ck * T:(ck + 1) * T])

              # --- qk^T = W_qk @ x^T, fp8 DoubleRow (W pre-scaled x32;
              # the x1024 on logits is divided out of the host mask) ---
              qkT = qkT_p.tile([128, 12, T], bf16, tag="qkT")
              for i in range(12):
                  pq = ps_ms.tile([128, 2, 2, SPAD], f32,
                                  tag="ms", name="pq").rearrange(
                                      "p a b f -> p (a b f)")[:, :392]
                  for kp in range(3):
                      nc.tensor.matmul(
                          pq[:], wqk_sb[:, kp, :, i * 128:(i + 1) * 128],
                          xT8[:, kp, :, :T], start=(kp == 0), stop=(kp == 2),
                          perf_mode=DR)
                  nc.scalar.activation(qkT[:, i, :T], pq[:], AF.Identity,
                                       bias=bqk_sb[:, i:i + 1])

              # --- V token-major, per (batch, segment) slices ---
              vts = []
              for b in range(BPCHUNK):
                  for seg in range(2):
                      soff = b * N + seg * SEG
                      vt = v_p.tile([128, H, D + 1], bf16, tag="v")
                      pv = [ps_vp.tile([128, 384], f32, tag="vp",
                                       name="pv")[:SEG]
                            for _ in range(2)]
                      for j in range(6):
                          lhs = xT[:, j, soff:soff + SEG]
                          for half in range(2):
                              nc.tensor.matmul(
                                  pv[half], lhs,
                                  wv_sb[:, j, half * 384:(half + 1) * 384],
                                  start=(j == 0), stop=(j == 5))
                      for half in range(2):
                          nc.any.tensor_tensor(
                              vt[:SEG, half * 6:(half + 1) * 6, :D],
                              pv[half].rearrange("p (h d) -> p h d", d=D),
                              bv_sb[:SEG, half * 384:(half + 1) * 384]
                              .rearrange("p (h d) -> p h d", d=D),
                              OP.add)
                      nc.gpsimd.memset(vt[:SEG, :, D:D + 1], 1.0)
                      vts.append(vt)

              # --- attention, head-pair structured ---
              # Odd heads live at partition base 64 of qkT, so their K=64
              # S^T matmuls auto-derive tile_position=(64,0); emitting the
              # even/odd matmuls back-to-back lets the PE run them
              # concurrently in different row groups. PV of pair p-1 is
              # emitted after S of pair p so the PE never waits on exp.
              ot = ot_p.tile([128, 6, T], bf16, tag="ot")
              scr = dram_p.tile([24, N], f32, name="scr")

              def emit_pv(pend):
                  b, j, pt = pend
                  po = ps_o.tile([D + 1, 2, SPAD], f32, tag="o")
                  for hp in range(2):
                      for seg in range(2):
                          nc.tensor.matmul(
                              po[:, hp, :N],
                              vts[b * 2 + seg][:SEG, 2 * j + hp, :],
                              pt[:SEG, hp, seg, :],
                              start=(seg == 0), stop=(seg == 1))
                  rt = rc_p.tile([1, 2, N], f32, tag="rc")
                  nc.any.tensor_copy(rt[:], po[D:D + 1, :, :N])
                  # scratch rows 2h+b for h = 2j, 2j+1  ->  rows (4j+b), (4j+2+b)
                  nc.sync.dma_start(
                      scr[4 * j + b: 4 * j + b + 3: 2, :], rt[:, :, :])
                  for hp in range(2):
                      nc.any.tensor_copy(
                          ot[hp * 64:(hp + 1) * 64, j, b * N:(b + 1) * N],
                          po[:D, hp, :N])

              pend = None
              pair_no = 0
              for b in range(BPCHUNK):
                  for j in range(6):
                      if b == 1 and j == 0:
                          if pend is not None:
                              emit_pv(pend)
                              pend = None
                          emit_norms(ot, scr, 0)
                      if prev is not None and pair_no % 3 == 2:
                          emit_proj_tile(*prev, pair_no // 3)
                      pair_no += 1
                      s_t = ps_ms.tile([128, 2, 2, SPAD], f32, tag="ms",
                                       name="st")
                      # S^T matmuls: interleave even/odd head (row groups
                      # 0/64). hp-major slices so the two concurrently
                      # running hp matmuls write different PSUM banks.
                      for seg in range(2):
                          moff = seg * SEG
                          for hp in range(2):
                              pb = hp * 64
                              k_ap = qkT[pb:pb + 64, 6 + j,
                                         b * N + moff: b * N + moff + SEG]
                              q_ap = qkT[pb:pb + 64, j, b * N: b * N + N]
                              nc.tensor.matmul(
                                  s_t[:SEG, hp, seg, :N], k_ap, q_ap,
                                  start=True, stop=True)
                      if pend is not None:
                          emit_pv(pend)
                      pt = p_p.tile([128, 2, 2, N], bf16, tag="p")
                      nc.vector.tensor_tensor(
                          pt[:SEG].rearrange("p a b f -> p (a b) f"),
                          s_t[:SEG].rearrange("p a b f -> p (a b) f")
                          [:, :, :N],
                          m_sb[:, 2 * j:2 * j + 2]
                          .rearrange("p a b f -> p (a b) f"), OP.mult)
                      pt_flat = pt[:SEG].rearrange("p a b f -> p (a b f)")
                      nc.scalar.activation(pt_flat, pt_flat, AF.Exp)
                      pend = (b, j, pt)
              emit_pv(pend)

              emit_norms(ot, scr, 1)

              prev = (ot, ck)
              first = False
          emit_proj(*prev)
          prev = None

    nc.compile()
    return nc


def _get_nc(repeat=1, loop=0):
    key = ("nc", repeat, loop)
    if key not in _CACHE:
        _CACHE[key] = _build(repeat, loop)
    return _CACHE[key]


def _prep_shared(W_qkv, b_qkv, att_mask, W_proj, b_proj):
    W_qkv = np.asarray(W_qkv, np.float32)
    W_proj = np.asarray(W_proj, np.float32)
    b_qkv = np.asarray(b_qkv, np.float32)
    b_proj = np.asarray(b_proj, np.float32)
    att_mask = np.asarray(att_mask, np.float32)
    sig = (SCALE / (QK_WSCALE * QK_WSCALE)) / (1.0 + np.exp(-att_mask))
    maskT = sig.transpose(0, 2, 1)                   # [H, m, n]
    m98 = np.ascontiguousarray(
        maskT.reshape(H, 2, SEG, N).transpose(2, 0, 1, 3))  # [98, H, 2, n]
    return {
        "wqkT": np.ascontiguousarray(
            (W_qkv[:2 * C] * QK_WSCALE).T).astype(_np_fp8()),
        "wvT": np.ascontiguousarray(W_qkv[2 * C:].T).astype(_np_bf16()),
        "wpT": np.ascontiguousarray(W_proj.T).astype(_np_bf16()),
        "bqk": np.ascontiguousarray(
            b_qkv[:2 * C].reshape(12, 128).T * QK_WSCALE),
        "bv": np.ascontiguousarray(b_qkv[2 * C:].reshape(1, C)),
        "bp": np.ascontiguousarray(b_proj.reshape(1, C)),
        "mask98": m98.astype(_np_bf16()),
    }


def _make_runner(nc, in_maps, n_cores, fetch=True):
    """Compile once and stage all inputs on device; repeat calls only
    re-execute on device (no host->device transfers, no re-tracing).
    fetch=False skips the device->host copy of y (for timing)."""
    import jax
    from jax.experimental.shard_map import shard_map
    from jax.sharding import Mesh, NamedSharding, PartitionSpec

    import concourse.bass2jax as b2j
    import concourse.mybir as mybir

    b2j.install_neuronx_cc_hook()
    partition_name = (nc.partition_id_tensor.name
                      if nc.partition_id_tensor else None)
    in_names, out_names, out_avals, zero_outs = [], [], [], []
    for alloc in nc.m.functions[0].allocations:
        if not isinstance(alloc, mybir.MemoryLocationSet):
            continue
        name = alloc.memorylocations[0].name
        if alloc.kind == "ExternalInput":
            if name != partition_name:
                in_names.append(name)
        elif alloc.kind == "ExternalOutput":
            shape = tuple(alloc.tensor_shape)
            dtype = mybir.dt.np(alloc.dtype)
            out_names.append(name)
            out_avals.append(jax.core.ShapedArray(shape, dtype))
            zero_outs.append(np.zeros(shape, dtype))
    n_params = len(in_names)
    in_names_all = list(in_names) + list(out_names)
    if partition_name is not None:
        in_names_all.append(partition_name)

    def _body(*args):
        operands = list(args)
        if partition_name is not None:
            operands.append(b2j.partition_id_tensor())
        outs = b2j._bass_exec_p.bind(
            *operands,
            out_avals=tuple(out_avals),
            in_names=tuple(in_names_all),
            out_names=tuple(out_names),
            lowering_input_output_aliases=(),
            sim_require_finite=True,
            sim_require_nnan=True,
            nc=nc,
        )
        return tuple(outs)

    devices = jax.devices()[:n_cores]
    mesh = Mesh(np.asarray(devices), ("core",))
    spec = NamedSharding(mesh, PartitionSpec("core"))
    n_all = n_params + len(out_names)
    sharded = jax.jit(
        shard_map(_body, mesh=mesh,
                  in_specs=(PartitionSpec("core"),) * n_all,
                  out_specs=(PartitionSpec("core"),) * len(out_names),
                  check_rep=False),
        keep_unused=True)

    per_core = [[np.asarray(m[name]) for name in in_names] for m in in_maps]
    dev_args = [
        jax.device_put(
            np.concatenate([per_core[c][i] for c in range(n_cores)], axis=0),
            spec)
        for i in range(n_params)
    ]
    dev_args += [
        jax.device_put(
            np.zeros((n_cores * z.shape[0], *z.shape[1:]), z.dtype), spec)
        for z in zero_outs
    ]

    yi = out_names.index("y")

    def run():
        out = sharded(*dev_args)
        jax.block_until_ready(out)
        if not fetch:
            return None
        return np.asarray(out[yi])

    return run


def kernel(x, W_qkv, b_qkv, att_mask, W_proj, b_proj):
    in_maps = _make_in_maps({"x": x, "W_qkv": W_qkv, "b_qkv": b_qkv,
                             "att_mask": att_mask, "W_proj": W_proj,
                             "b_proj": b_proj})
    try:
        cached = _CACHE.get("runner")
        if cached is not None:
            old_maps, run = cached
            same = all(
                np.array_equal(old_maps[c][k], in_maps[c][k])
                for c in range(NCORES) for k in in_maps[c])
            if not same:
                cached = None
        if cached is None:
            run = _make_runner(_get_nc(), in_maps, NCORES)
            _CACHE["runner"] = (in_maps, run)
        y_cat = run()
        out = y_cat.reshape(NCORES, BPC, N, C)
    except Exception:
        _CACHE.pop("runner", None)
        from concourse.bass_utils import run_bass_kernel_spmd
        res = run_bass_kernel_spmd(_get_nc(), in_maps,
                                   core_ids=list(range(NCORES)))
        out = np.stack([res.results[c]["y"].reshape(BPC, N, C)
                        for c in range(NCORES)])
    return out.reshape(B, N, C).astype(np.float32)


def _make_in_maps(inputs):
    """Build the per-core input maps from the full (unsharded) inputs."""
    x = np.asarray(inputs["x"], np.float32)
    shared = _prep_shared(inputs["W_qkv"], inputs["b_qkv"],
                          inputs["att_mask"], inputs["W_proj"],
                          inputs["b_proj"])
    in_maps = []
    for c in range(NCORES):
        m = dict(shared)
        xtc = np.ascontiguousarray(
            x[c * BPC:(c + 1) * BPC].reshape(BPC * N, C).T)
        m["xt"] = xtc.astype(_np_bf16())
        m["xt8"] = xtc.astype(_np_fp8())
        in_maps.append(m)
    return in_maps


# revision 35
# speedup vs baseline: 1.3628x; 1.0154x over previous
"""Trainium2 Bass kernel for nn_Attention_90486370992549.

Learned-sigmoid-mask multi-head attention:
  qkv = x @ W_qkv.T + b_qkv
  attn = softmax((q k^T / sqrt(D)) * sigmoid(att_mask))
  out  = (attn @ v) @ W_proj.T + b_proj

Sharding: data-parallel over batch across 8 NeuronCores (16 batches/core).
All matmuls run in bf16 (inputs rounded host-side / on copy; f32 PSUM
accumulation), which gives the full 1 cycle/row PE rate at any moving
free-dim size and FWL weight loads.

Per-core plan, processed in 8 chunks of 2 batches (392 tokens):
  - x^T is pre-transposed host-side and DMA'd straight into SBUF (bf16),
    so the PE does no transposes at all
  - qk^T = (W_qk x^T) in outc-major layout -> per-head q,k are D-major
  - V in token-major layout (separate matmul, x^T as stationary),
    key dim split 196 = 98 + 98 so both segments use the same partition
    count and the mask-multiply/exp fuse into one op per head pair
  - per (batch, head-pair): S^T = k^T q for (seg, hp) in one PSUM tile,
    multiply by sigmoid-mask (pre-scaled, transposed, host-side, bf16),
    exp (no max-subtract: logits are ~N(0, 0.16)), PV with a ones-column
    in V giving the softmax denominator as row 64 of the PSUM output
  - denominators are copied out per pair, broadcast across partitions via
    a DRAM round-trip DMA, then a single approximate-reciprocal (custom
    DVE op, ~51 ULP) per batch; normalization runs on GPSIMD
  - proj uses O^T as the matmul stationary -> token-major output, no final
    transpose; proj of chunk k-1 is emitted inside chunk k so the in-order
    PE queue never stalls on the normalization barrier.
"""

import numpy as np

B, N, C, H, D = 128, 196, 768, 12, 64
SCALE = D ** -0.5
NCORES = 8
BPC = B // NCORES              # batches per core
BPCHUNK = 2                    # batches per chunk
NCHUNK = BPC // BPCHUNK        # 8 chunks
T = BPCHUNK * N                # 392 tokens per chunk
TOK_TILES = [(0, 128), (128, 128), (256, 128), (384, 8)]
SEG = 98                       # key-dim segment (196 = 2 x 98)
SPAD = 256                     # PSUM slice stride (bank alignment)
RECIP_APPROX = True
PROJ_AT = (3, 6, 8, 10)        # pairs at which prev-chunk proj tiles go out

_CACHE = {}


def _np_bf16():
    import ml_dtypes
    return np.dtype(ml_dtypes.bfloat16)


def _np_fp8():
    import ml_dtypes
    return np.dtype(ml_dtypes.float8_e4m3)


QK_WSCALE = 32.0               # fp8 weight pre-scale (host); mask absorbs it


def _build(repeat=1, loop=0):
    from contextlib import ExitStack

    import concourse.bacc as bacc
    import concourse.bass as bass
    import concourse.mybir as mybir
    from concourse.tile import TileContext

    f32 = mybir.dt.float32
    bf16 = mybir.dt.bfloat16
    fp8 = mybir.dt.float8e4
    AF = mybir.ActivationFunctionType
    OP = mybir.AluOpType
    DR = mybir.MatmulPerfMode.DoubleRow

    nc = bacc.Bacc("TRN2", target_bir_lowering=False, debug=False,
                   num_devices=NCORES)
    xt = nc.dram_tensor("xt", [C, BPC * N], bf16, kind="ExternalInput")
    xt8 = nc.dram_tensor("xt8", [C, BPC * N], fp8, kind="ExternalInput")
    wqkT = nc.dram_tensor("wqkT", [C, 2 * C], fp8, kind="ExternalInput")
    wvT = nc.dram_tensor("wvT", [C, C], bf16, kind="ExternalInput")
    wpT = nc.dram_tensor("wpT", [C, C], bf16, kind="ExternalInput")
    bqk = nc.dram_tensor("bqk", [128, 12], f32, kind="ExternalInput")
    bv = nc.dram_tensor("bv", [1, C], f32, kind="ExternalInput")
    bp = nc.dram_tensor("bp", [1, C], f32, kind="ExternalInput")
    mask98 = nc.dram_tensor("mask98", [SEG, H, 2, N], bf16,
                            kind="ExternalInput")
    y = nc.dram_tensor("y", [BPC * N, C], f32, kind="ExternalOutput")

    xt_r = xt.rearrange("(j p) t -> p j t", p=128)
    xt8_r = xt8.rearrange("(a b p) t -> p a b t", a=3, b=2)
    TP8 = 400                   # fp8 x^T tile pad (16B-aligned ko stride)

    with TileContext(nc) as tc, ExitStack() as ctx:
        singles = ctx.enter_context(tc.tile_pool(name="singles", bufs=1))
        xT_p = ctx.enter_context(tc.tile_pool(name="xT", bufs=2))
        qkT_p = ctx.enter_context(tc.tile_pool(name="qkT", bufs=2))
        v_p = ctx.enter_context(tc.tile_pool(name="v", bufs=4))
        ot_p = ctx.enter_context(tc.tile_pool(name="ot", bufs=2))
        p_p = ctx.enter_context(tc.tile_pool(name="p", bufs=4))
        y_p = ctx.enter_context(tc.tile_pool(name="y", bufs=2))
        rc_p = ctx.enter_context(tc.tile_pool(name="rc", bufs=2))
        bc_p = ctx.enter_context(tc.tile_pool(name="bc", bufs=1))
        dram_p = ctx.enter_context(tc.tile_pool(name="dram", bufs=2,
                                                space="DRAM"))
        ps_ms = ctx.enter_context(tc.tile_pool(name="psms", bufs=2,
                                               space="PSUM"))
        ps_o = ctx.enter_context(tc.tile_pool(name="pso", bufs=2,
                                              space="PSUM"))
        ps_vp = ctx.enter_context(tc.tile_pool(name="psvp", bufs=2,
                                               space="PSUM"))

        # --- prefetch chunk-0 x^T before the big weight DMAs so the
        # qk matmuls can start immediately ---
        xT8_0 = xT_p.tile([128, 3, 2, TP8], fp8, tag="xT8", name="xT8_0")
        nc.sync.dma_start(xT8_0[:, :, :, :T], xt8_r[:, :, :, 0:T])
        xT0 = xT_p.tile([128, 6, T], bf16, tag="xT", name="xT0")
        nc.sync.dma_start(xT0[:], xt_r[:, :, 0:T])

        # --- resident weights / constants ---
        wqk_sb = singles.tile([128, 3, 2, 2 * C], fp8)
        _wqk_r = wqkT.rearrange("(a b p) n -> p a b n", a=3, b=2)
        _splits = [0, 128, 384, 768, 1152, 1536]
        for _a in range(len(_splits) - 1):
            nc.sync.dma_start(wqk_sb[:, :, :, _splits[_a]:_splits[_a + 1]],
                              _wqk_r[:, :, :, _splits[_a]:_splits[_a + 1]])
        bqk_sb = singles.tile([128, 12], f32)
        nc.sync.dma_start(bqk_sb[:], bqk[:])
        wv_sb = singles.tile([128, 6, C], bf16)
        nc.sync.dma_start(wv_sb[:], wvT.rearrange("(ko p) n -> p ko n", p=128))
        bv_sb = singles.tile([128, C], f32)
        bv_ap = bv.ap()
        nc.sync.dma_start(bv_sb[:], bass.AP(
            tensor=bv_ap.tensor, offset=bv_ap.offset,
            ap=[[0, 128], bv_ap.ap[1]]))
        m_sb = singles.tile([SEG, H, 2, N], bf16)
        nc.sync.dma_start(m_sb[:], mask98[:])
        wp_sb = singles.tile([128, 6, C], bf16)
        nc.sync.dma_start(wp_sb[:], wpT.rearrange("(ko p) n -> p ko n", p=128))
        bp_sb = singles.tile([128, C], f32)
        bp_ap = bp.ap()
        nc.sync.dma_start(bp_sb[:], bass.AP(
            tensor=bp_ap.tensor, offset=bp_ap.offset,
            ap=[[0, 128], bp_ap.ap[1]]))

        def emit_proj_tile(ot, ck, ti):
            off, rows = TOK_TILES[ti]
            ph = [ps_vp.tile([128, 384], f32, tag="vp", name="ph")[:rows]
                  for _ in range(2)]
            for j in range(6):
                lhs = ot[:, j, off:off + rows]
                for half in range(2):
                    nc.tensor.matmul(
                        ph[half], lhs,
                        wp_sb[:, j, half * 384:(half + 1) * 384],
                        start=(j == 0), stop=(j == 5))
            y_sb = y_p.tile([128, C], f32, tag="y")
            for half in range(2):
                nc.any.tensor_tensor(
                    y_sb[:rows, half * 384:(half + 1) * 384],
                    ph[half], bp_sb[:rows, half * 384:(half + 1) * 384],
                    OP.add)
            nc.sync.dma_start(
                y[ck * T + off: ck * T + off + rows, :], y_sb[:rows])

        def emit_proj(ot, ck):
            for ti in range(len(TOK_TILES)):
                emit_proj_tile(ot, ck, ti)

        def emit_norms(ot, scr, b):
            """Broadcast den for batch b via DRAM round-trip, take the
            reciprocal once on the broadcast tile (custom DVE approx op),
            then normalize O^T columns of batch b in-place (on GPSIMD)."""
            scr_ap = scr[:]
            bc = bc_p.tile([128, 6, N], f32, tag="bc", name=f"bc{b}")
            for hp in range(2):
                nc.sync.dma_start(
                    bc[hp * 64:(hp + 1) * 64],
                    bass.AP(tensor=scr_ap.tensor,
                            offset=scr_ap.offset + (2 * hp + b) * N,
                            ap=[[0, 64], [4 * N, 6], [1, N]]))
            bcr = bc_p.tile([128, 6, N], f32, tag="bcr", name=f"bcr{b}")
            if RECIP_APPROX:
                nc.vector.reciprocal_approx_fast(bcr[:], bc[:])
            else:
                nc.vector.reciprocal(bcr[:], bc[:])
            for j in range(6):
                sl = ot[:, j, b * N:(b + 1) * N]
                nc.gpsimd.tensor_tensor(sl, sl, bcr[:, j, :], OP.mult)

        from contextlib import nullcontext
        loop_cm = tc.For_i(0, loop, 1) if loop else nullcontext()
        prev = None
        first = not loop
        with loop_cm:
          for ck in [c for _ in range(repeat) for c in range(NCHUNK)]:
              # --- x^T arrives pre-transposed from DRAM ---
              if ck == 0 and first:
                  xT8 = xT8_0
                  xT = xT0
              else:
                  xT8 = xT_p.tile([128, 3, 2, TP8], fp8, tag="xT8")
                  nc.sync.dma_start(xT8[:, :, :, :T],
                                    xt8_r[:, :, :, ck * T:(ck + 1) * T])
                  xT = xT_p.tile([128, 6, T], bf16, tag="xT")
                  nc.sync.dma_start(xT[:], xt_r[:, :, ck * T:(ck + 1) * T])

              # --- qk^T columns (fp8 DoubleRow; W pre-scaled x32, the
              # x1024 on logits divided out of the host mask) and V are
              # emitted interleaved with the attention pairs below, so the
              # in-order PE queue always has matmul work while DVE/ACT
              # drain the per-pair mask/exp/copy chain. ---
              qkT = qkT_p.tile([128, 12, T], bf16, tag="qkT")

              def emit_qk(i):
                  pq = ps_ms.tile([128, 2, 2, SPAD], f32,
                                  tag="ms", name="pq").rearrange(
                                      "p a b f -> p (a b f)")[:, :392]
                  for kp in range(3):
                      nc.tensor.matmul(
                          pq[:], wqk_sb[:, kp, :, i * 128:(i + 1) * 128],
                          xT8[:, kp, :, :T], start=(kp == 0), stop=(kp == 2),
                          perf_mode=DR)
                  nc.scalar.activation(qkT[:, i, :T], pq[:], AF.Identity,
                                       bias=bqk_sb[:, i:i + 1])

              vts = {}

              def emit_v(b):
                  for seg in range(2):
                      soff = b * N + seg * SEG
                      vt = v_p.tile([128, H, D + 1], bf16, tag="v",
                                    name="vt")
                      pv = [ps_vp.tile([128, 384], f32, tag="vp",
                                       name="pv")[:SEG]
                            for _ in range(2)]
                      for j in range(6):
                          lhs = xT[:, j, soff:soff + SEG]
                          for half in range(2):
                              nc.tensor.matmul(
                                  pv[half], lhs,
                                  wv_sb[:, j, half * 384:(half + 1) * 384],
                                  start=(j == 0), stop=(j == 5))
                      for half in range(2):
                          nc.any.tensor_tensor(
                              vt[:SEG, half * 6:(half + 1) * 6, :D],
                              pv[half].rearrange("p (h d) -> p h d", d=D),
                              bv_sb[:SEG, half * 384:(half + 1) * 384]
                              .rearrange("p (h d) -> p h d", d=D),
                              OP.add)
                      nc.gpsimd.memset(vt[:SEG, :, D:D + 1], 1.0)
                      vts[b * 2 + seg] = vt

              # --- attention, head-pair structured ---
              # Odd heads live at partition base 64 of qkT, so their K=64
              # S^T matmuls auto-derive tile_position=(64,0); emitting the
              # even/odd matmuls back-to-back lets the PE run them
              # concurrently in different row groups. PV of pair p-1 is
              # emitted after S of pair p so the PE never waits on exp.
              ot = ot_p.tile([128, 6, T], bf16, tag="ot")
              scr = dram_p.tile([24, N], f32, name="scr")

              def emit_pv(pend):
                  b, j, pt = pend
                  po = ps_o.tile([D + 1, 2, SPAD], f32, tag="o")
                  for hp in range(2):
                      for seg in range(2):
                          nc.tensor.matmul(
                              po[:, hp, :N],
                              vts[b * 2 + seg][:SEG, 2 * j + hp, :],
                              pt[:SEG, hp, seg, :],
                              start=(seg == 0), stop=(seg == 1))
                  rt = rc_p.tile([1, 2, N], f32, tag="rc")
                  nc.any.tensor_copy(rt[:], po[D:D + 1, :, :N])
                  # scratch rows 2h+b for h = 2j, 2j+1  ->  rows (4j+b), (4j+2+b)
                  nc.sync.dma_start(
                      scr[4 * j + b: 4 * j + b + 3: 2, :], rt[:, :, :])
                  for hp in range(2):
                      nc.any.tensor_copy(
                          ot[hp * 64:(hp + 1) * 64, j, b * N:(b + 1) * N],
                          po[:D, hp, :N])

              emit_qk(0)
              emit_qk(6)
              emit_v(0)

              pend = None
              pair_no = 0
              for b in range(BPCHUNK):
                  for j in range(6):
                      if b == 0:
                          if j < 5:
                              emit_qk(j + 1)
                              emit_qk(7 + j)
                          if j == 3:
                              emit_v(1)
                      if b == 1 and j == 0:
                          if pend is not None:
                              emit_pv(pend)
                              pend = None
                          emit_norms(ot, scr, 0)
                      if prev is not None and pair_no in PROJ_AT:
                          emit_proj_tile(*prev, PROJ_AT.index(pair_no))
                      pair_no += 1
                      s_t = ps_ms.tile([128, 2, 2, SPAD], f32, tag="ms",
                                       name="st")
                      # S^T matmuls: interleave even/odd head (row groups
                      # 0/64). hp-major slices so the two concurrently
                      # running hp matmuls write different PSUM banks.
                      for seg in range(2):
                          moff = seg * SEG
                          for hp in range(2):
                              pb = hp * 64
                              k_ap = qkT[pb:pb + 64, 6 + j,
                                         b * N + moff: b * N + moff + SEG]
                              q_ap = qkT[pb:pb + 64, j, b * N: b * N + N]
                              nc.tensor.matmul(
                                  s_t[:SEG, hp, seg, :N], k_ap, q_ap,
                                  start=True, stop=True)
                      if pend is not None:
                          emit_pv(pend)
                      pt = p_p.tile([128, 2, 2, N], bf16, tag="p")
                      nc.vector.tensor_tensor(
                          pt[:SEG].rearrange("p a b f -> p (a b) f"),
                          s_t[:SEG].rearrange("p a b f -> p (a b) f")
                          [:, :, :N],
                          m_sb[:, 2 * j:2 * j + 2]
                          .rearrange("p a b f -> p (a b) f"), OP.mult)
                      pt_flat = pt[:SEG].rearrange("p a b f -> p (a b f)")
                      nc.scalar.activation(pt_flat, pt_flat, AF.Exp)
                      pend = (b, j, pt)
              emit_pv(pend)

              emit_norms(ot, scr, 1)

              prev = (ot, ck)
              first = False
          emit_proj(*prev)
          prev = None

    nc.compile()
    return nc


def _get_nc(repeat=1, loop=0):
    key = ("nc", repeat, loop)
    if key not in _CACHE:
        _CACHE[key] = _build(repeat, loop)
    return _CACHE[key]


def _prep_shared(W_qkv, b_qkv, att_mask, W_proj, b_proj):
    W_qkv = np.asarray(W_qkv, np.float32)
    W_proj = np.asarray(W_proj, np.float32)
    b_qkv = np.asarray(b_qkv, np.float32)
    b_proj = np.asarray(b_proj, np.float32)
    att_mask = np.asarray(att_mask, np.float32)
    sig = (SCALE / (QK_WSCALE * QK_WSCALE)) / (1.0 + np.exp(-att_mask))
    maskT = sig.transpose(0, 2, 1)                   # [H, m, n]
    m98 = np.ascontiguousarray(
        maskT.reshape(H, 2, SEG, N).transpose(2, 0, 1, 3))  # [98, H, 2, n]
    return {
        "wqkT": np.ascontiguousarray(
            (W_qkv[:2 * C] * QK_WSCALE).T).astype(_np_fp8()),
        "wvT": np.ascontiguousarray(W_qkv[2 * C:].T).astype(_np_bf16()),
        "wpT": np.ascontiguousarray(W_proj.T).astype(_np_bf16()),
        "bqk": np.ascontiguousarray(
            b_qkv[:2 * C].reshape(12, 128).T * QK_WSCALE),
        "bv": np.ascontiguousarray(b_qkv[2 * C:].reshape(1, C)),
        "bp": np.ascontiguousarray(b_proj.reshape(1, C)),
        "mask98": m98.astype(_np_bf16()),
    }


def _make_runner(nc, in_maps, n_cores, fetch=True):
    """Compile once and stage all inputs on device; repeat calls only
    re-execute on device (no host->device transfers, no re-tracing).
    fetch=False skips the device->host copy of y (for timing)."""
    import jax
    from jax.experimental.shard_map import shard_map
    from jax.sharding import Mesh, NamedSharding, PartitionSpec

    import concourse.bass2jax as b2j
    import concourse.mybir as mybir

    b2j.install_neuronx_cc_hook()
    partition_name = (nc.partition_id_tensor.name
                      if nc.partition_id_tensor else None)
    in_names, out_names, out_avals, zero_outs = [], [], [], []
    for alloc in nc.m.functions[0].allocations:
        if not isinstance(alloc, mybir.MemoryLocationSet):
            continue
        name = alloc.memorylocations[0].name
        if alloc.kind == "ExternalInput":
            if name != partition_name:
                in_names.append(name)
        elif alloc.kind == "ExternalOutput":
            shape = tuple(alloc.tensor_shape)
            dtype = mybir.dt.np(alloc.dtype)
            out_names.append(name)
            out_avals.append(jax.core.ShapedArray(shape, dtype))
            zero_outs.append(np.zeros(shape, dtype))
    n_params = len(in_names)
    in_names_all = list(in_names) + list(out_names)
    if partition_name is not None:
        in_names_all.append(partition_name)

    def _body(*args):
        operands = list(args)
        if partition_name is not None:
            operands.append(b2j.partition_id_tensor())
        outs = b2j._bass_exec_p.bind(
            *operands,
            out_avals=tuple(out_avals),
            in_names=tuple(in_names_all),
            out_names=tuple(out_names),
            lowering_input_output_aliases=(),
            sim_require_finite=True,
            sim_require_nnan=True,
            nc=nc,
        )
        return tuple(outs)

    devices = jax.devices()[:n_cores]
    mesh = Mesh(np.asarray(devices), ("core",))
    spec = NamedSharding(mesh, PartitionSpec("core"))
    n_all = n_params + len(out_names)
    sharded = jax.jit(
        shard_map(_body, mesh=mesh,
                  in_specs=(PartitionSpec("core"),) * n_all,
                  out_specs=(PartitionSpec("core"),) * len(out_names),
                  check_rep=False),
        keep_unused=True)

    per_core = [[np.asarray(m[name]) for name in in_names] for m in in_maps]
    dev_args = [
        jax.device_put(
            np.concatenate([per_core[c][i] for c in range(n_cores)], axis=0),
            spec)
        for i in range(n_params)
    ]
    dev_args += [
        jax.device_put(
            np.zeros((n_cores * z.shape[0], *z.shape[1:]), z.dtype), spec)
        for z in zero_outs
    ]

    yi = out_names.index("y")

    def run():
        out = sharded(*dev_args)
        jax.block_until_ready(out)
        if not fetch:
            return None
        return np.asarray(out[yi])

    return run


def kernel(x, W_qkv, b_qkv, att_mask, W_proj, b_proj):
    in_maps = _make_in_maps({"x": x, "W_qkv": W_qkv, "b_qkv": b_qkv,
                             "att_mask": att_mask, "W_proj": W_proj,
                             "b_proj": b_proj})
    try:
        cached = _CACHE.get("runner")
        if cached is not None:
            old_maps, run = cached
            same = all(
                np.array_equal(old_maps[c][k], in_maps[c][k])
                for c in range(NCORES) for k in in_maps[c])
            if not same:
                cached = None
        if cached is None:
            run = _make_runner(_get_nc(), in_maps, NCORES)
            _CACHE["runner"] = (in_maps, run)
        y_cat = run()
        out = y_cat.reshape(NCORES, BPC, N, C)
    except Exception:
        _CACHE.pop("runner", None)
        from concourse.bass_utils import run_bass_kernel_spmd
        res = run_bass_kernel_spmd(_get_nc(), in_maps,
                                   core_ids=list(range(NCORES)))
        out = np.stack([res.results[c]["y"].reshape(BPC, N, C)
                        for c in range(NCORES)])
    return out.reshape(B, N, C).astype(np.float32)


def _make_in_maps(inputs):
    """Build the per-core input maps from the full (unsharded) inputs."""
    x = np.asarray(inputs["x"], np.float32)
    shared = _prep_shared(inputs["W_qkv"], inputs["b_qkv"],
                          inputs["att_mask"], inputs["W_proj"],
                          inputs["b_proj"])
    in_maps = []
    for c in range(NCORES):
        m = dict(shared)
        xtc = np.ascontiguousarray(
            x[c * BPC:(c + 1) * BPC].reshape(BPC * N, C).T)
        m["xt"] = xtc.astype(_np_bf16())
        m["xt8"] = xtc.astype(_np_fp8())
        in_maps.append(m)
    return in_maps


# revision 43
# speedup vs baseline: 1.4021x; 1.0289x over previous
"""Trainium2 Bass kernel for nn_Attention_90486370992549.

Learned-sigmoid-mask multi-head attention:
  qkv = x @ W_qkv.T + b_qkv
  attn = softmax((q k^T / sqrt(D)) * sigmoid(att_mask))
  out  = (attn @ v) @ W_proj.T + b_proj

Sharding: data-parallel over batch across 8 NeuronCores (16 batches/core).
All matmuls run in bf16 (inputs rounded host-side / on copy; f32 PSUM
accumulation), which gives the full 1 cycle/row PE rate at any moving
free-dim size and FWL weight loads.

Per-core plan, processed in 8 chunks of 2 batches (392 tokens):
  - x^T is pre-transposed host-side and DMA'd straight into SBUF (bf16),
    so the PE does no transposes at all
  - qk^T = (W_qk x^T) in outc-major layout -> per-head q,k are D-major
  - V in token-major layout (separate matmul, x^T as stationary),
    key dim split 196 = 98 + 98 so both segments use the same partition
    count and the mask-multiply/exp fuse into one op per head pair
  - per (batch, head-pair): S^T = k^T q for (seg, hp) in one PSUM tile,
    multiply by sigmoid-mask (pre-scaled, transposed, host-side, bf16),
    exp (no max-subtract: logits are ~N(0, 0.16)), PV with a ones-column
    in V giving the softmax denominator as row 64 of the PSUM output
  - denominators are copied out per pair, broadcast across partitions via
    a DRAM round-trip DMA, then a single approximate-reciprocal (custom
    DVE op, ~51 ULP) per batch; normalization runs on GPSIMD
  - proj uses O^T as the matmul stationary -> token-major output, no final
    transpose; proj of chunk k-1 is emitted inside chunk k so the in-order
    PE queue never stalls on the normalization barrier.
"""

import numpy as np

B, N, C, H, D = 128, 196, 768, 12, 64
SCALE = D ** -0.5
NCORES = 8
BPC = B // NCORES              # batches per core
BPCHUNK = 2                    # batches per chunk
NCHUNK = BPC // BPCHUNK        # 8 chunks
T = BPCHUNK * N                # 392 tokens per chunk
TOK_TILES = [(0, 128), (128, 128), (256, 128), (384, 8)]
SEG = 98                       # key-dim segment (196 = 2 x 98)
SPAD = 256                     # PSUM slice stride (bank alignment)
RECIP_APPROX = True
PROJ_AT = (3, 6, 8, 10)        # pairs at which prev-chunk proj tiles go out

_CACHE = {}


def _np_bf16():
    import ml_dtypes
    return np.dtype(ml_dtypes.bfloat16)


def _np_fp8():
    import ml_dtypes
    return np.dtype(ml_dtypes.float8_e4m3)


QK_WSCALE = 32.0               # fp8 weight pre-scale (host); mask absorbs it


def _build(repeat=1, loop=0):
    from contextlib import ExitStack

    import concourse.bacc as bacc
    import concourse.bass as bass
    import concourse.mybir as mybir
    from concourse.tile import TileContext

    f32 = mybir.dt.float32
    bf16 = mybir.dt.bfloat16
    fp8 = mybir.dt.float8e4
    AF = mybir.ActivationFunctionType
    OP = mybir.AluOpType
    DR = mybir.MatmulPerfMode.DoubleRow

    nc = bacc.Bacc("TRN2", target_bir_lowering=False, debug=False,
                   num_devices=NCORES)
    # x^T and W_qk are pre-swizzled host-side into partition-major chunked
    # layouts so every per-chunk DMA is one contiguous run per partition.
    xt = nc.dram_tensor("xt", [128, NCHUNK, 6, T], bf16,
                        kind="ExternalInput")
    xt8 = nc.dram_tensor("xt8", [128, NCHUNK, 3, 2, 400], fp8,
                         kind="ExternalInput")
    wqkT = nc.dram_tensor("wqkT", [128, 12, 3, 2, 128], fp8,
                          kind="ExternalInput")
    wvT = nc.dram_tensor("wvT", [C, C], bf16, kind="ExternalInput")
    wpT = nc.dram_tensor("wpT", [C, C], bf16, kind="ExternalInput")
    bqk = nc.dram_tensor("bqk", [128, 12], f32, kind="ExternalInput")
    bv = nc.dram_tensor("bv", [1, C], f32, kind="ExternalInput")
    bp = nc.dram_tensor("bp", [1, C], f32, kind="ExternalInput")
    mask98 = nc.dram_tensor("mask98", [SEG, H, 2, N], bf16,
                            kind="ExternalInput")
    y = nc.dram_tensor("y", [BPC * N, C], f32, kind="ExternalOutput")

    TP8 = 400                   # fp8 x^T tile pad (16B-aligned ko stride)

    with TileContext(nc) as tc, ExitStack() as ctx:
        singles = ctx.enter_context(tc.tile_pool(name="singles", bufs=1))
        xT_p = ctx.enter_context(tc.tile_pool(name="xT", bufs=2))
        qkT_p = ctx.enter_context(tc.tile_pool(name="qkT", bufs=2))
        v_p = ctx.enter_context(tc.tile_pool(name="v", bufs=4))
        ot_p = ctx.enter_context(tc.tile_pool(name="ot", bufs=2))
        p_p = ctx.enter_context(tc.tile_pool(name="p", bufs=4))
        y_p = ctx.enter_context(tc.tile_pool(name="y", bufs=2))
        rc_p = ctx.enter_context(tc.tile_pool(name="rc", bufs=2))
        bc_p = ctx.enter_context(tc.tile_pool(name="bc", bufs=1))
        dram_p = ctx.enter_context(tc.tile_pool(name="dram", bufs=2,
                                                space="DRAM"))
        ps_ms = ctx.enter_context(tc.tile_pool(name="psms", bufs=2,
                                               space="PSUM"))
        ps_o = ctx.enter_context(tc.tile_pool(name="pso", bufs=2,
                                              space="PSUM"))
        ps_vp = ctx.enter_context(tc.tile_pool(name="psvp", bufs=2,
                                               space="PSUM"))

        # --- prefetch chunk-0 x^T and the first qk pair-column's weights
        # before the big weight DMAs so the qk matmuls start immediately ---
        xT8_0 = xT_p.tile([128, 3, 2, TP8], fp8, tag="xT8", name="xT8_0")
        nc.sync.dma_start(xT8_0[:], xt8[:, 0])
        xT0 = xT_p.tile([128, 6, T], bf16, tag="xT", name="xT0")
        nc.sync.dma_start(xT0[:], xt[:, 0])

        # --- resident weights / constants ---
        wqk_sb = singles.tile([128, 12, 3, 2, 128], fp8)
        for i0, i1 in ((0, 1), (6, 7), (1, 6), (7, 12)):
            nc.sync.dma_start(wqk_sb[:, i0:i1], wqkT[:, i0:i1])
        bqk_sb = singles.tile([128, 12], f32)
        nc.sync.dma_start(bqk_sb[:], bqk[:])
        wv_sb = singles.tile([128, 6, C], bf16)
        nc.sync.dma_start(wv_sb[:], wvT.rearrange("(ko p) n -> p ko n", p=128))
        bv_sb = singles.tile([128, C], f32)
        bv_ap = bv.ap()
        nc.sync.dma_start(bv_sb[:], bass.AP(
            tensor=bv_ap.tensor, offset=bv_ap.offset,
            ap=[[0, 128], bv_ap.ap[1]]))
        m_sb = singles.tile([SEG, H, 2, N], bf16)
        nc.sync.dma_start(m_sb[:], mask98[:])
        wp_sb = singles.tile([128, 6, C], bf16)
        nc.sync.dma_start(wp_sb[:], wpT.rearrange("(ko p) n -> p ko n", p=128))
        bp_sb = singles.tile([128, C], f32)
        bp_ap = bp.ap()
        nc.sync.dma_start(bp_sb[:], bass.AP(
            tensor=bp_ap.tensor, offset=bp_ap.offset,
            ap=[[0, 128], bp_ap.ap[1]]))

        def emit_proj_tile(ot, ck, ti):
            off, rows = TOK_TILES[ti]
            ph = [ps_vp.tile([128, 384], f32, tag="vp", name="ph")[:rows]
                  for _ in range(2)]
            for j in range(6):
                lhs = ot[:, j, off:off + rows]
                for half in range(2):
                    nc.tensor.matmul(
                        ph[half], lhs,
                        wp_sb[:, j, half * 384:(half + 1) * 384],
                        start=(j == 0), stop=(j == 5))
            y_sb = y_p.tile([128, C], f32, tag="y")
            for half in range(2):
                nc.any.tensor_tensor(
                    y_sb[:rows, half * 384:(half + 1) * 384],
                    ph[half], bp_sb[:rows, half * 384:(half + 1) * 384],
                    OP.add)
            nc.sync.dma_start(
                y[ck * T + off: ck * T + off + rows, :], y_sb[:rows])

        def emit_proj(ot, ck):
            for ti in range(len(TOK_TILES)):
                emit_proj_tile(ot, ck, ti)

        def emit_norms(ot, scr, b):
            """Broadcast den for batch b via DRAM round-trip, reciprocal on
            the broadcast tile (custom DVE approx op), then normalize O^T
            columns of batch b in-place (on GPSIMD). Pipelined per head
            pair so the chain starts as soon as each pair's dens land."""
            scr_ap = scr[:]
            bc = bc_p.tile([128, 6, N], f32, tag="bc", name=f"bc{b}")
            bcr = bc_p.tile([128, 6, N], f32, tag="bcr", name=f"bcr{b}")
            for j in range(6):
                for hp in range(2):
                    nc.sync.dma_start(
                        bc[hp * 64:(hp + 1) * 64, j],
                        bass.AP(tensor=scr_ap.tensor,
                                offset=scr_ap.offset + (4 * j + 2 * hp + b) * N,
                                ap=[[0, 64], [1, N]]))
                if RECIP_APPROX:
                    nc.vector.reciprocal_approx_fast(bcr[:, j], bc[:, j])
                else:
                    nc.vector.reciprocal(bcr[:, j], bc[:, j])
                sl = ot[:, j, b * N:(b + 1) * N]
                nc.gpsimd.tensor_tensor(sl, sl, bcr[:, j, :], OP.mult)

        from contextlib import nullcontext
        loop_cm = tc.For_i(0, loop, 1) if loop else nullcontext()
        prev = None
        first = not loop
        with loop_cm:
          for ck in [c for _ in range(repeat) for c in range(NCHUNK)]:
              # --- x^T arrives pre-transposed from DRAM ---
              if ck == 0 and first:
                  xT8 = xT8_0
                  xT = xT0
              else:
                  xT8 = xT_p.tile([128, 3, 2, TP8], fp8, tag="xT8")
                  nc.sync.dma_start(xT8[:], xt8[:, ck])
                  xT = xT_p.tile([128, 6, T], bf16, tag="xT")
                  nc.sync.dma_start(xT[:], xt[:, ck])

              # --- qk^T columns (fp8 DoubleRow; W pre-scaled x32, the
              # x1024 on logits divided out of the host mask) and V are
              # emitted interleaved with the attention pairs below, so the
              # in-order PE queue always has matmul work while DVE/ACT
              # drain the per-pair mask/exp/copy chain. ---
              qkT = qkT_p.tile([128, 12, T], bf16, tag="qkT")

              def emit_qk(i):
                  pq = ps_ms.tile([128, 2, 2, SPAD], f32,
                                  tag="ms", name="pq").rearrange(
                                      "p a b f -> p (a b f)")[:, :392]
                  for kp in range(3):
                      nc.tensor.matmul(
                          pq[:], wqk_sb[:, i, kp],
                          xT8[:, kp, :, :T], start=(kp == 0), stop=(kp == 2),
                          perf_mode=DR)
                  nc.any.tensor_scalar_add(qkT[:, i, :T], pq[:],
                                           bqk_sb[:, i:i + 1])

              vts = {}

              def emit_v(b):
                  for seg in range(2):
                      soff = b * N + seg * SEG
                      vt = v_p.tile([128, H, D + 1], bf16, tag="v",
                                    name="vt")
                      pv = [ps_vp.tile([128, 384], f32, tag="vp",
                                       name="pv")[:SEG]
                            for _ in range(2)]
                      for j in range(6):
                          lhs = xT[:, j, soff:soff + SEG]
                          for half in range(2):
                              nc.tensor.matmul(
                                  pv[half], lhs,
                                  wv_sb[:, j, half * 384:(half + 1) * 384],
                                  start=(j == 0), stop=(j == 5))
                      for half in range(2):
                          nc.any.tensor_tensor(
                              vt[:SEG, half * 6:(half + 1) * 6, :D],
                              pv[half].rearrange("p (h d) -> p h d", d=D),
                              bv_sb[:SEG, half * 384:(half + 1) * 384]
                              .rearrange("p (h d) -> p h d", d=D),
                              OP.add)
                      nc.gpsimd.memset(vt[:SEG, :, D:D + 1], 1.0)
                      vts[b * 2 + seg] = vt

              # --- attention, head-pair structured ---
              # Odd heads live at partition base 64 of qkT, so their K=64
              # S^T matmuls auto-derive tile_position=(64,0); emitting the
              # even/odd matmuls back-to-back lets the PE run them
              # concurrently in different row groups. PV of pair p-1 is
              # emitted after S of pair p so the PE never waits on exp.
              ot = ot_p.tile([128, 6, T], bf16, tag="ot")
              scr = dram_p.tile([24, N], f32, name="scr")

              def emit_pv(pend):
                  b, j, pt = pend
                  po = ps_o.tile([D + 1, 2, SPAD], f32, tag="o")
                  for hp in range(2):
                      for seg in range(2):
                          nc.tensor.matmul(
                              po[:, hp, :N],
                              vts[b * 2 + seg][:SEG, 2 * j + hp, :],
                              pt[:SEG, hp, seg, :],
                              start=(seg == 0), stop=(seg == 1))
                  rt = rc_p.tile([1, 2, N], f32, tag="rc")
                  nc.any.tensor_copy(rt[:], po[D:D + 1, :, :N])
                  # scratch rows 2h+b for h = 2j, 2j+1  ->  rows (4j+b), (4j+2+b)
                  nc.sync.dma_start(
                      scr[4 * j + b: 4 * j + b + 3: 2, :], rt[:, :, :])
                  for hp in range(2):
                      nc.any.tensor_copy(
                          ot[hp * 64:(hp + 1) * 64, j, b * N:(b + 1) * N],
                          po[:D, hp, :N])

              emit_qk(0)
              emit_qk(6)
              emit_v(0)

              pend = None
              pair_no = 0
              for b in range(BPCHUNK):
                  for j in range(6):
                      if b == 0:
                          if j < 5:
                              emit_qk(j + 1)
                              emit_qk(7 + j)
                          if j == 3:
                              emit_v(1)
                      if b == 1 and j == 0:
                          if pend is not None:
                              emit_pv(pend)
                              pend = None
                          emit_norms(ot, scr, 0)
                      if prev is not None and pair_no in PROJ_AT:
                          emit_proj_tile(*prev, PROJ_AT.index(pair_no))
                      pair_no += 1
                      s_t = ps_ms.tile([128, 2, 2, SPAD], f32, tag="ms",
                                       name="st")
                      # S^T matmuls: interleave even/odd head (row groups
                      # 0/64). hp-major slices so the two concurrently
                      # running hp matmuls write different PSUM banks.
                      for seg in range(2):
                          moff = seg * SEG
                          for hp in range(2):
                              pb = hp * 64
                              k_ap = qkT[pb:pb + 64, 6 + j,
                                         b * N + moff: b * N + moff + SEG]
                              q_ap = qkT[pb:pb + 64, j, b * N: b * N + N]
                              nc.tensor.matmul(
                                  s_t[:SEG, hp, seg, :N], k_ap, q_ap,
                                  start=True, stop=True)
                      if pend is not None:
                          emit_pv(pend)
                      pt = p_p.tile([128, 2, 2, N], bf16, tag="p")
                      nc.vector.tensor_tensor(
                          pt[:SEG].rearrange("p a b f -> p (a b) f"),
                          s_t[:SEG].rearrange("p a b f -> p (a b) f")
                          [:, :, :N],
                          m_sb[:, 2 * j:2 * j + 2]
                          .rearrange("p a b f -> p (a b) f"), OP.mult)
                      pt_flat = pt[:SEG].rearrange("p a b f -> p (a b f)")
                      nc.scalar.activation(pt_flat, pt_flat, AF.Exp)
                      pend = (b, j, pt)
              emit_pv(pend)

              emit_norms(ot, scr, 1)

              prev = (ot, ck)
              first = False
          emit_proj(*prev)
          prev = None

    nc.compile()
    return nc


def _get_nc(repeat=1, loop=0):
    key = ("nc", repeat, loop)
    if key not in _CACHE:
        _CACHE[key] = _build(repeat, loop)
    return _CACHE[key]


def _prep_shared(W_qkv, b_qkv, att_mask, W_proj, b_proj):
    W_qkv = np.asarray(W_qkv, np.float32)
    W_proj = np.asarray(W_proj, np.float32)
    b_qkv = np.asarray(b_qkv, np.float32)
    b_proj = np.asarray(b_proj, np.float32)
    att_mask = np.asarray(att_mask, np.float32)
    sig = (SCALE / (QK_WSCALE * QK_WSCALE)) / (1.0 + np.exp(-att_mask))
    maskT = sig.transpose(0, 2, 1)                   # [H, m, n]
    m98 = np.ascontiguousarray(
        maskT.reshape(H, 2, SEG, N).transpose(2, 0, 1, 3))  # [98, H, 2, n]
    # W_qk: [C, 2C] -> partition-major [128, 12, 3, 2, 128]
    # (row c = (a*2 + b)*128 + p, col n = i*128 + col)
    wqk8 = np.ascontiguousarray(
        (W_qkv[:2 * C] * QK_WSCALE).T.reshape(3, 2, 128, 12, 128)
        .transpose(2, 3, 0, 1, 4)).astype(_np_fp8())
    return {
        "wqkT": wqk8,
        "wvT": np.ascontiguousarray(W_qkv[2 * C:].T).astype(_np_bf16()),
        "wpT": np.ascontiguousarray(W_proj.T).astype(_np_bf16()),
        "bqk": np.ascontiguousarray(
            b_qkv[:2 * C].reshape(12, 128).T * QK_WSCALE),
        "bv": np.ascontiguousarray(b_qkv[2 * C:].reshape(1, C)),
        "bp": np.ascontiguousarray(b_proj.reshape(1, C)),
        "mask98": m98.astype(_np_bf16()),
    }


def _make_runner(nc, in_maps, n_cores, fetch=True):
    """Compile once and stage all inputs on device; repeat calls only
    re-execute on device (no host->device transfers, no re-tracing).
    fetch=False skips the device->host copy of y (for timing)."""
    import jax
    from jax.experimental.shard_map import shard_map
    from jax.sharding import Mesh, NamedSharding, PartitionSpec

    import concourse.bass2jax as b2j
    import concourse.mybir as mybir

    b2j.install_neuronx_cc_hook()
    partition_name = (nc.partition_id_tensor.name
                      if nc.partition_id_tensor else None)
    in_names, out_names, out_avals, zero_outs = [], [], [], []
    for alloc in nc.m.functions[0].allocations:
        if not isinstance(alloc, mybir.MemoryLocationSet):
            continue
        name = alloc.memorylocations[0].name
        if alloc.kind == "ExternalInput":
            if name != partition_name:
                in_names.append(name)
        elif alloc.kind == "ExternalOutput":
            shape = tuple(alloc.tensor_shape)
            dtype = mybir.dt.np(alloc.dtype)
            out_names.append(name)
            out_avals.append(jax.core.ShapedArray(shape, dtype))
            zero_outs.append(np.zeros(shape, dtype))
    n_params = len(in_names)
    in_names_all = list(in_names) + list(out_names)
    if partition_name is not None:
        in_names_all.append(partition_name)

    def _body(*args):
        operands = list(args)
        if partition_name is not None:
            operands.append(b2j.partition_id_tensor())
        outs = b2j._bass_exec_p.bind(
            *operands,
            out_avals=tuple(out_avals),
            in_names=tuple(in_names_all),
            out_names=tuple(out_names),
            lowering_input_output_aliases=(),
            sim_require_finite=True,
            sim_require_nnan=True,
            nc=nc,
        )
        return tuple(outs)

    devices = jax.devices()[:n_cores]
    mesh = Mesh(np.asarray(devices), ("core",))
    spec = NamedSharding(mesh, PartitionSpec("core"))
    n_all = n_params + len(out_names)
    sharded = jax.jit(
        shard_map(_body, mesh=mesh,
                  in_specs=(PartitionSpec("core"),) * n_all,
                  out_specs=(PartitionSpec("core"),) * len(out_names),
                  check_rep=False),
        keep_unused=True)

    per_core = [[np.asarray(m[name]) for name in in_names] for m in in_maps]
    dev_args = [
        jax.device_put(
            np.concatenate([per_core[c][i] for c in range(n_cores)], axis=0),
            spec)
        for i in range(n_params)
    ]
    dev_args += [
        jax.device_put(
            np.zeros((n_cores * z.shape[0], *z.shape[1:]), z.dtype), spec)
        for z in zero_outs
    ]

    yi = out_names.index("y")

    def run():
        out = sharded(*dev_args)
        jax.block_until_ready(out)
        if not fetch:
            return None
        return np.asarray(out[yi])

    return run


def kernel(x, W_qkv, b_qkv, att_mask, W_proj, b_proj):
    in_maps = _make_in_maps({"x": x, "W_qkv": W_qkv, "b_qkv": b_qkv,
                             "att_mask": att_mask, "W_proj": W_proj,
                             "b_proj": b_proj})
    try:
        cached = _CACHE.get("runner")
        if cached is not None:
            old_maps, run = cached
            same = all(
                np.array_equal(old_maps[c][k], in_maps[c][k])
                for c in range(NCORES) for k in in_maps[c])
            if not same:
                cached = None
        if cached is None:
            run = _make_runner(_get_nc(), in_maps, NCORES)
            _CACHE["runner"] = (in_maps, run)
        y_cat = run()
        out = y_cat.reshape(NCORES, BPC, N, C)
    except Exception:
        _CACHE.pop("runner", None)
        from concourse.bass_utils import run_bass_kernel_spmd
        res = run_bass_kernel_spmd(_get_nc(), in_maps,
                                   core_ids=list(range(NCORES)))
        out = np.stack([res.results[c]["y"].reshape(BPC, N, C)
                        for c in range(NCORES)])
    return out.reshape(B, N, C).astype(np.float32)


def _make_in_maps(inputs):
    """Build the per-core input maps from the full (unsharded) inputs."""
    x = np.asarray(inputs["x"], np.float32)
    shared = _prep_shared(inputs["W_qkv"], inputs["b_qkv"],
                          inputs["att_mask"], inputs["W_proj"],
                          inputs["b_proj"])
    in_maps = []
    for c in range(NCORES):
        m = dict(shared)
        xtc = x[c * BPC:(c + 1) * BPC].reshape(BPC * N, C).T  # [C, 3136]
        # [128, NCHUNK, 6, T]: row c = j*128 + p, col t = ck*T + tau
        m["xt"] = np.ascontiguousarray(
            xtc.reshape(6, 128, NCHUNK, T).transpose(1, 2, 0, 3)
        ).astype(_np_bf16())
        # [128, NCHUNK, 3, 2, 400]: row c = (a*2 + b)*128 + p, padded tau
        x8 = np.zeros((128, NCHUNK, 3, 2, 400), np.float32)
        x8[..., :T] = xtc.reshape(3, 2, 128, NCHUNK, T).transpose(2, 3, 0, 1, 4)
        m["xt8"] = x8.astype(_np_fp8())
        in_maps.append(m)
    return in_maps
